# revision 24
# baseline (speedup 1.0000x reference)
"""GCN (2-layer graph convolution, symmetric norm) on 8 TRN2 NeuronCores.

Design (graph/data parallel per sharding hint, optimized for the TRN2 cost
model: per-DMA fixed costs, single SWDGE gather queue, bf16 tensor engine):

 - Host preprocessing (indices/layout only): degrees, edge sort, a
   bin-packing of receivers into 8*98 blocks of 128 slots balancing edge
   count (so every block needs exactly KS=6 gather groups of 128 edges),
   fp8 one-hot segment-sum masks, int16 local gather indices, and per-core
   halo tables (each core receives only the node rows its edges reference,
   split into block-ranges so local ids fit in int16).
 - Phase A (node-sharded): h1s = lrelu(x@W1+b1) @ W2 + b2 in feature-major
   chunks; bf16 after the first matmul; writes the bf16 h1s table in
   (partition, block) row order.
 - Phase B (edge-sharded): per receiver block, ONE dma_gather fetches
   6x128 sender rows (bf16) from the core's halo table; fp8 one-hot masks
   (streamed on the idle SP queue) segment-sum via PE matmuls;
   lrelu(agg*invr) @ Wd + bd scaled by invs -> bf16 h2s table.
 - Phase C: same aggregation over the h2s halo (rows padded to 256B for
   dma_gather), then softmax via Exp on the scalar engine (logits are O(10);
   no max-subtract needed) + DVE row-sum/reciprocal.
Host does only index preprocessing, layout permutation, and shard (halo)
assembly between phases.
"""

import numpy as np
import ml_dtypes

N = 100000
E = 600000
D = 128
C = 40
NCORES = 8
NS = N // NCORES          # 12500 nodes per core
P = 128
NB = (NS + P - 1) // P    # 98 receiver blocks per core
NPAD = NB * P             # 12544
NBINS = NCORES * NB       # 784
TROWS = 32768             # halo table rows (int16-addressable)

BF16 = ml_dtypes.bfloat16
FP8 = ml_dtypes.float8_e4m3


# ---------------------------------------------------------------- host side

def preprocess(x, senders, receivers, W1, b1, W2, b2, Wd, bd):
    x = np.asarray(x, np.float32)
    senders = np.asarray(senders, np.int64)
    receivers = np.asarray(receivers, np.int64)
    pre = {
        "W1": np.ascontiguousarray(np.asarray(W1, np.float32)),
        "W2": np.ascontiguousarray(np.asarray(W2, np.float32)),
        "Wd": np.ascontiguousarray(np.asarray(Wd, np.float32)),
        "b1row": np.asarray(b1, np.float32).reshape(1, D),
        "b2row": np.asarray(b2, np.float32).reshape(1, D),
        "bdrow": np.asarray(bd, np.float32).reshape(1, C),
    }

    deg_s = np.bincount(senders, minlength=N).astype(np.float32)
    deg_r = np.bincount(receivers, minlength=N).astype(np.float32)
    inv_s = (1.0 / np.sqrt(np.maximum(deg_s, 1.0))).astype(np.float32)
    inv_r = (1.0 / np.sqrt(np.maximum(deg_r, 1.0))).astype(np.float32)

    # --- bin-pack receivers into NBINS bins of <=128 slots, balancing edges
    import heapq
    order = np.argsort(-deg_r, kind="stable")
    heap = [(0.0, b) for b in range(NBINS)]
    heapq.heapify(heap)
    slots_used = np.zeros(NBINS, np.int32)
    assign_bin = np.empty(N, np.int32)
    slot_p = np.empty(N, np.int32)
    for n in order:
        while True:
            load, b = heapq.heappop(heap)
            if slots_used[b] < P:
                break
        assign_bin[n] = b
        slot_p[n] = slots_used[b]
        slots_used[b] += 1
        heapq.heappush(heap, (load + float(deg_r[n]), b))

    bin_load = np.bincount(assign_bin[receivers], minlength=NBINS)
    KS = int(np.ceil(bin_load.max() / P))
    pre["KS"] = KS

    # --- table-row maps
    # phase A table order: node n -> row (n//NS)*NPAD + (n%NS % P)*NB + (n%NS//P)
    nn = np.arange(N, dtype=np.int64)
    loc = nn % NS
    rowA = (nn // NS) * NPAD + (loc % P) * NB + (loc // P)
    # phase B table order: node n -> its aggregation slot row
    rowB = (assign_bin.astype(np.int64) // NB) * NPAD + \
        slot_p.astype(np.int64) * NB + (assign_bin.astype(np.int64) % NB)
    pre["rowA"] = rowA
    pre["rowB"] = rowB

    # --- edge slot assignment: per (core, block), k-major flat slot list
    ebin = assign_bin[receivers]
    eorder = np.argsort(ebin, kind="stable")
    ebin_s = ebin[eorder].astype(np.int64)
    esend = senders[eorder]
    eq = slot_p[receivers][eorder].astype(np.int64)          # local recv slot
    binstarts = np.searchsorted(ebin_s, np.arange(NBINS))
    pos = np.arange(E, dtype=np.int64) - binstarts[ebin_s]
    ek = pos // P
    elane = pos % P
    ec = ebin_s // NB
    eb = ebin_s % NB

    KSLOT = KS * P
    sendnode = np.full((NCORES, NB, KSLOT), -1, np.int64)
    sendnode[ec, eb, ek * P + elane] = esend
    pre["sendnode"] = sendnode

    mask = np.zeros((NCORES, P, NB * KS * P), np.uint8)
    mask[ec, elane, (eb * KS + ek) * P + eq] = 0x38          # fp8e4m3 1.0
    pre["mask"] = mask.view(FP8)

    # --- halo split: block ranges with <=TROWS-1 unique senders each
    nsplit = 2
    while True:
        bounds = np.linspace(0, NB, nsplit + 1).astype(int)
        splits = [(int(bounds[i]), int(bounds[i + 1])) for i in range(nsplit)]
        gids = []        # [c][s] -> node ids in halo table order
        ok = True
        for c in range(NCORES):
            row = []
            for lo, hi in splits:
                nodes = sendnode[c, lo:hi]
                uniq = np.unique(nodes[nodes >= 0])
                if uniq.shape[0] > TROWS - 1:
                    ok = False
                row.append(uniq)
            gids.append(row)
            if not ok:
                break
        if ok:
            break
        nsplit += 1
    pre["splits"] = splits
    pre["gids"] = gids

    # --- int16 packed gather indices (wrapped in 16 partitions, replicated)
    idx16 = np.zeros((NCORES, P, NB * KS * 8), np.int16)
    for c in range(NCORES):
        for s, (lo, hi) in enumerate(splits):
            uniq = gids[c][s]
            nodes = sendnode[c, lo:hi]                       # [nb, KSLOT]
            local = np.zeros(nodes.shape, np.int64)
            valid = nodes >= 0
            local[valid] = np.searchsorted(uniq, nodes[valid])
            # wrap each block's flat list: w[i, t] = flat[t*16 + i]
            nb = hi - lo
            w = local.reshape(nb, KSLOT // 16, 16).transpose(0, 2, 1)
            w = w.reshape(nb, 16, KSLOT // 16)
            idx16[c, :, lo * KS * 8:hi * KS * 8] = np.tile(
                w, (1, 8, 1)).transpose(1, 0, 2).reshape(P, nb * KS * 8)
    pre["idx16"] = idx16

    # --- per-slot scale vectors
    node_at = np.full((NCORES, NB, P), -1, np.int64)
    ab = assign_bin.astype(np.int64)
    node_at[ab // NB, ab % NB, slot_p] = nn
    pre["node_at"] = node_at
    safe = np.maximum(node_at, 0)
    invr_blk = np.where(node_at >= 0, inv_r[safe], 1.0).astype(np.float32)
    invs_blk = np.where(node_at >= 0, inv_s[safe], 1.0).astype(np.float32)
    pre["invr_blk"] = np.ascontiguousarray(invr_blk.transpose(0, 2, 1))  # [c,P,NB]
    pre["invs_blk"] = np.ascontiguousarray(invs_blk.transpose(0, 2, 1))

    # phase-A-order inv_s: [c, P, NB]
    invsA = np.ones((NCORES, NPAD), np.float32)
    invsA[:, :NS] = inv_s.reshape(NCORES, NS)
    pre["invsA"] = np.ascontiguousarray(
        invsA.reshape(NCORES, NB, P).transpose(0, 2, 1))

    # x transposed per core: [c, D, NPAD]
    xT = np.zeros((NCORES, D, NPAD), np.float32)
    xT[:, :, :NS] = x.reshape(NCORES, NS, D).transpose(0, 2, 1)
    pre["xT"] = xT

    pre["ones512"] = np.ones((1, 512), np.float32)
    pre["inv_s"] = inv_s
    pre["inv_r"] = inv_r
    pre["bias_nz"] = bool(np.any(pre["b1row"]) or np.any(pre["b2row"])
                          or np.any(pre["bdrow"]))
    return pre


def build_halo_tabs(pre, table_full, rowmap, width):
    """Per-core halo tables [nsplit][TROWS, P] bf16 from a full table."""
    tabs = []
    tf = np.asarray(table_full)
    for c in range(NCORES):
        row = []
        for s in range(len(pre["splits"])):
            gid = pre["gids"][c][s]
            t = np.zeros((TROWS, P), BF16)
            t[:gid.shape[0], :width] = tf[rowmap[gid], :width]
            row.append(t)
        tabs.append(row)
    return tabs


def maps_a(pre):
    return [
        {"xT": pre["xT"][c], "W1": pre["W1"], "W2": pre["W2"],
         "b1row": pre["b1row"], "b2row": pre["b2row"],
         "ones512": pre["ones512"], "invsA": pre["invsA"][c]}
        for c in range(NCORES)
    ]


def maps_b(pre, tabs):
    return [
        {**{f"tab{s}": tabs[c][s] for s in range(len(pre["splits"]))},
         "idx": pre["idx16"][c], "mask": pre["mask"][c],
         "invr": pre["invr_blk"][c], "invs": pre["invs_blk"][c],
         "Wd": pre["Wd"], "bdrow": pre["bdrow"], "ones512": pre["ones512"]}
        for c in range(NCORES)
    ]


def maps_c(pre, tabs):
    return [
        {**{f"tab{s}": tabs[c][s] for s in range(len(pre["splits"]))},
         "idx": pre["idx16"][c], "mask": pre["mask"][c],
         "invr": pre["invr_blk"][c]}
        for c in range(NCORES)
    ]


# ------------------------------------------------------------- bass kernels

def _chunks():
    out = []
    b = 0
    while b < NB:
        w = min(4, NB - b)
        out.append((b, w))
        b += w
    return out


def _build_phase_a(bias_nz=True):
    from concourse import bacc, mybir, tile

    f32 = mybir.dt.float32
    bf16 = mybir.dt.bfloat16
    nc = bacc.Bacc("TRN2", target_bir_lowering=False, debug=False)
    xT_ext = nc.declare_dram_parameter("xT", [D, NPAD], f32, isOutput=False)
    w1_ext = nc.declare_dram_parameter("W1", [D, D], f32, isOutput=False)
    w2_ext = nc.declare_dram_parameter("W2", [D, D], f32, isOutput=False)
    b1_ext = nc.declare_dram_parameter("b1row", [1, D], f32, isOutput=False)
    b2_ext = nc.declare_dram_parameter("b2row", [1, D], f32, isOutput=False)
    ones_ext = nc.declare_dram_parameter("ones512", [1, 512], f32, isOutput=False)
    invsA_ext = nc.declare_dram_parameter("invsA", [P, NB], f32, isOutput=False)
    out_ext = nc.declare_dram_parameter("h1s", [P, NB * D], bf16, isOutput=True)

    with tile.TileContext(nc) as tc:
        with (
            tc.tile_pool(name="const", bufs=1) as cp,
            tc.tile_pool(name="xin", bufs=4) as xp,
            tc.tile_pool(name="work", bufs=4) as sp,
            tc.tile_pool(name="stg", bufs=4) as gp,
            tc.tile_pool(name="ps1", bufs=3, space="PSUM") as pp1,
            tc.tile_pool(name="ps2", bufs=3, space="PSUM") as pp2,
        ):
            w1 = cp.tile([D, D], dtype=f32)
            nc.sync.dma_start(out=w1[:], in_=w1_ext[:])
            w2f = cp.tile([D, D], dtype=f32)
            nc.sync.dma_start(out=w2f[:], in_=w2_ext[:])
            w2 = cp.tile([D, D], dtype=bf16)
            nc.vector.tensor_copy(w2[:], w2f[:])
            b1 = cp.tile([1, D], dtype=f32)
            nc.sync.dma_start(out=b1[:], in_=b1_ext[:])
            b2 = cp.tile([1, D], dtype=f32)
            nc.sync.dma_start(out=b2[:], in_=b2_ext[:])
            ones = cp.tile([1, 512], dtype=f32)
            nc.sync.dma_start(out=ones[:], in_=ones_ext[:])
            invsA = cp.tile([P, NB], dtype=f32)
            nc.sync.dma_start(out=invsA[:], in_=invsA_ext[:])

            for ci, (b0, w) in enumerate(_chunks()):
                cw = w * P
                c0 = b0 * P
                xt = xp.tile([D, cw], dtype=f32)
                ldq = nc.sync if ci % 2 == 0 else nc.gpsimd
                ldq.dma_start(out=xt[:], in_=xT_ext[:, c0:c0 + cw])
                # y1 = x@W1 (+ b1), feature-major [D, cw]
                ps1 = pp1.tile([P, cw], dtype=f32, space="PSUM")
                nc.tensor.matmul(out=ps1[:], lhsT=w1[:], rhs=xt[:],
                                 start=True, stop=not bias_nz)
                if bias_nz:
                    nc.tensor.matmul(out=ps1[:], lhsT=b1[:],
                                     rhs=ones[:, :cw], start=False, stop=True)
                # lrelu: t01 = 0.01*y1 (Act), z1 = max(y1, t01) (DVE)
                t01 = sp.tile([P, cw], dtype=bf16)
                nc.scalar.mul(t01[:], ps1[:], 0.01)
                z1 = sp.tile([P, cw], dtype=bf16)
                nc.vector.tensor_tensor(out=z1[:], in0=ps1[:], in1=t01[:],
                                        op=mybir.AluOpType.max)
                # per 128-node block: y2 = z1_blk.T @ W2 (+ b2), node-major
                stg = gp.tile([P, cw], dtype=bf16)
                for j in range(w):
                    ps2 = pp2.tile([P, D], dtype=f32, space="PSUM")
                    nc.tensor.matmul(out=ps2[:], lhsT=z1[:, j * P:(j + 1) * P],
                                     rhs=w2[:], start=True, stop=not bias_nz)
                    if bias_nz:
                        nc.tensor.matmul(out=ps2[:], lhsT=b2[:],
                                         rhs=ones[:, :D], start=False, stop=True)
                    bcol = b0 + j
                    dst = stg[:, j * P:(j + 1) * P]
                    if j == 1:
                        nc.scalar.activation(
                            out=dst, in_=ps2[:],
                            func=mybir.ActivationFunctionType.Copy,
                            bias=0.0, scale=invsA[:, bcol:bcol + 1])
                    else:
                        sv = invsA[:, bcol:bcol + 1].to_broadcast([P, D])
                        nc.vector.tensor_tensor(out=dst, in0=ps2[:], in1=sv,
                                                op=mybir.AluOpType.mult)
                wrq = nc.gpsimd if ci % 2 == 0 else nc.sync
                wrq.dma_start(out=out_ext[:, c0:c0 + cw], in_=stg[:])
    nc.finalize()
    return nc


def _split_of(splits):
    which = np.empty(NB, np.int32)
    for s, (lo, hi) in enumerate(splits):
        which[lo:hi] = s
    return which


def _build_phase_b(KS, splits, bias_nz=True):
    from concourse import bacc, mybir, tile
    from concourse.masks import make_identity

    f32 = mybir.dt.float32
    bf16 = mybir.dt.bfloat16
    fp8 = mybir.dt.float8e4
    i16 = mybir.dt.int16
    nc = bacc.Bacc("TRN2", target_bir_lowering=False, debug=False)
    tab_exts = [nc.declare_dram_parameter(f"tab{s}", [TROWS, P], bf16,
                                          isOutput=False)
                for s in range(len(splits))]
    idx_ext = nc.declare_dram_parameter("idx", [P, NB * KS * 8], i16,
                                        isOutput=False)
    mask_ext = nc.declare_dram_parameter("mask", [P, NB * KS * P], fp8,
                                         isOutput=False)
    invr_ext = nc.declare_dram_parameter("invr", [P, NB], f32, isOutput=False)
    invs_ext = nc.declare_dram_parameter("invs", [P, NB], f32, isOutput=False)
    wd_ext = nc.declare_dram_parameter("Wd", [D, C], f32, isOutput=False)
    bd_ext = nc.declare_dram_parameter("bdrow", [1, C], f32, isOutput=False)
    ones_ext = nc.declare_dram_parameter("ones512", [1, 512], f32, isOutput=False)
    out_ext = nc.declare_dram_parameter("h2s", [P, NB * C], bf16, isOutput=True)

    which = _split_of(splits)
    WGRP = 16

    with tile.TileContext(nc) as tc:
        with (
            tc.tile_pool(name="const", bufs=1) as cp,
            tc.tile_pool(name="gat", bufs=6) as gp,
            tc.tile_pool(name="msk", bufs=4) as mp,
            tc.tile_pool(name="work", bufs=6) as sp,
            tc.tile_pool(name="stg", bufs=2) as op,
            tc.tile_pool(name="psA", bufs=3, space="PSUM") as ppA,
            tc.tile_pool(name="psT", bufs=2, space="PSUM") as ppT,
            tc.tile_pool(name="psO", bufs=2, space="PSUM") as ppO,
        ):
            idx = cp.tile([P, NB * KS * 8], dtype=i16)
            nc.sync.dma_start(out=idx[:], in_=idx_ext[:])
            invr = cp.tile([P, NB], dtype=f32)
            nc.sync.dma_start(out=invr[:], in_=invr_ext[:])
            invs = cp.tile([P, NB], dtype=f32)
            nc.sync.dma_start(out=invs[:], in_=invs_ext[:])
            wdf = cp.tile([D, C], dtype=f32)
            nc.sync.dma_start(out=wdf[:], in_=wd_ext[:])
            wd = cp.tile([D, C], dtype=bf16)
            nc.vector.tensor_copy(wd[:], wdf[:])
            bd = cp.tile([1, C], dtype=f32)
            nc.sync.dma_start(out=bd[:], in_=bd_ext[:])
            ones = cp.tile([1, 512], dtype=f32)
            nc.sync.dma_start(out=ones[:], in_=ones_ext[:])
            identb = cp.tile([P, P], dtype=bf16)
            make_identity(nc, identb[:])

            mk = None
            stg = None
            for b in range(NB):
                if b % 2 == 0:
                    nmk = min(2, NB - b)
                    mk = mp.tile([P, nmk * KS * P], dtype=fp8)
                    nc.sync.dma_start(
                        out=mk[:],
                        in_=mask_ext[:, b * KS * P:(b + nmk) * KS * P])
                if b % WGRP == 0:
                    nw = min(WGRP, NB - b)
                    stg = op.tile([P, nw * C], dtype=bf16)
                g = gp.tile([P, KS, P], dtype=bf16)
                nc.gpsimd.dma_gather(
                    out_ap=g[:].bitcast(f32), in_ap=tab_exts[which[b]][:].bitcast(f32),
                    idxs_ap=idx[:, b * KS * 8:(b + 1) * KS * 8],
                    num_idxs=KS * P, num_idxs_reg=KS * P, elem_size=P // 2)
                psA = ppA.tile([P, D], dtype=f32, space="PSUM")
                mo = (b % 2) * KS * P
                for k in range(KS):
                    nc.tensor.matmul(
                        out=psA[:], lhsT=mk[:, mo + k * P:mo + (k + 1) * P],
                        rhs=g[:, k, :], start=(k == 0), stop=(k == KS - 1))
                # za = (agg * invr) bf16 (Act); zb = 0.01*za; h = max (DVE)
                za = sp.tile([P, D], dtype=bf16)
                nc.scalar.activation(out=za[:], in_=psA[:],
                                     func=mybir.ActivationFunctionType.Copy,
                                     bias=0.0, scale=invr[:, b:b + 1])
                zb = sp.tile([P, D], dtype=bf16)
                nc.vector.tensor_scalar_mul(zb[:], za[:], 0.01)
                h = sp.tile([P, D], dtype=bf16)
                nc.vector.tensor_tensor(out=h[:], in0=za[:], in1=zb[:],
                                        op=mybir.AluOpType.max)
                psT = ppT.tile([P, D], dtype=bf16, space="PSUM")
                nc.tensor.transpose(out=psT[:], in_=h[:], identity=identb[:])
                hT = sp.tile([P, D], dtype=bf16)
                if b % 2 == 0:
                    nc.scalar.copy(hT[:], psT[:])
                else:
                    nc.vector.tensor_copy(hT[:], psT[:])
                psO = ppO.tile([P, C], dtype=f32, space="PSUM")
                nc.tensor.matmul(out=psO[:], lhsT=hT[:], rhs=wd[:],
                                 start=True, stop=not bias_nz)
                if bias_nz:
                    nc.tensor.matmul(out=psO[:], lhsT=ones[:, :D], rhs=bd[:],
                                     start=False, stop=True)
                nc.vector.tensor_tensor(
                    out=stg[:, (b % WGRP) * C:(b % WGRP + 1) * C],
                    in0=psO[:], in1=invs[:, b:b + 1].to_broadcast([P, C]),
                    op=mybir.AluOpType.mult)
                if b % WGRP == WGRP - 1 or b == NB - 1:
                    w0 = (b // WGRP) * WGRP
                    nc.sync.dma_start(
                        out=out_ext[:, w0 * C:(b + 1) * C],
                        in_=stg[:, :(b + 1 - w0) * C])
    nc.finalize()
    return nc


def _build_phase_c(KS, splits):
    from concourse import bacc, mybir, tile

    f32 = mybir.dt.float32
    bf16 = mybir.dt.bfloat16
    fp8 = mybir.dt.float8e4
    i16 = mybir.dt.int16
    nc = bacc.Bacc("TRN2", target_bir_lowering=False, debug=False)
    tab_exts = [nc.declare_dram_parameter(f"tab{s}", [TROWS, P], bf16,
                                          isOutput=False)
                for s in range(len(splits))]
    idx_ext = nc.declare_dram_parameter("idx", [P, NB * KS * 8], i16,
                                        isOutput=False)
    mask_ext = nc.declare_dram_parameter("mask", [P, NB * KS * P], fp8,
                                         isOutput=False)
    invr_ext = nc.declare_dram_parameter("invr", [P, NB], f32, isOutput=False)
    out_ext = nc.declare_dram_parameter("res", [P, NB * C], f32, isOutput=True)

    which = _split_of(splits)
    WGRP = 16

    with tile.TileContext(nc) as tc:
        with (
            tc.tile_pool(name="const", bufs=1) as cp,
            tc.tile_pool(name="gat", bufs=6) as gp,
            tc.tile_pool(name="msk", bufs=4) as mp,
            tc.tile_pool(name="work", bufs=6) as sp,
            tc.tile_pool(name="stg", bufs=2) as op,
            tc.tile_pool(name="psC", bufs=6, space="PSUM") as ppC,
        ):
            idx = cp.tile([P, NB * KS * 8], dtype=i16)
            nc.sync.dma_start(out=idx[:], in_=idx_ext[:])
            invr = cp.tile([P, NB], dtype=f32)
            nc.sync.dma_start(out=invr[:], in_=invr_ext[:])

            mk = None
            stg = None
            for b in range(NB):
                if b % 2 == 0:
                    nmk = min(2, NB - b)
                    mk = mp.tile([P, nmk * KS * P], dtype=fp8)
                    nc.sync.dma_start(
                        out=mk[:],
                        in_=mask_ext[:, b * KS * P:(b + nmk) * KS * P])
                if b % WGRP == 0:
                    nw = min(WGRP, NB - b)
                    stg = op.tile([P, nw * C], dtype=f32)
                g = gp.tile([P, KS, P], dtype=bf16)
                nc.gpsimd.dma_gather(
                    out_ap=g[:].bitcast(f32), in_ap=tab_exts[which[b]][:].bitcast(f32),
                    idxs_ap=idx[:, b * KS * 8:(b + 1) * KS * 8],
                    num_idxs=KS * P, num_idxs_reg=KS * P, elem_size=P // 2)
                psC = ppC.tile([P, C], dtype=f32, space="PSUM")
                mo = (b % 2) * KS * P
                for k in range(KS):
                    nc.tensor.matmul(
                        out=psC[:], lhsT=mk[:, mo + k * P:mo + (k + 1) * P],
                        rhs=g[:, k, 0:C], start=(k == 0), stop=(k == KS - 1))
                # softmax: ex = exp(agg*invr) (logits O(10), no max-subtract)
                ex = sp.tile([P, C], dtype=bf16)
                nc.scalar.activation(out=ex[:], in_=psC[:],
                                     func=mybir.ActivationFunctionType.Exp,
                                     scale=invr[:, b:b + 1])
                dn = sp.tile([P, 1], dtype=f32)
                nc.vector.reduce_sum(dn[:], ex[:], axis=mybir.AxisListType.X)
                rd = sp.tile([P, 1], dtype=f32)
                nc.vector.reciprocal(rd[:], dn[:])
                nc.vector.tensor_tensor(
                    out=stg[:, (b % WGRP) * C:(b % WGRP + 1) * C],
                    in0=ex[:], in1=rd[:].to_broadcast([P, C]),
                    op=mybir.AluOpType.mult)
                if b % WGRP == WGRP - 1 or b == NB - 1:
                    w0 = (b // WGRP) * WGRP
                    nc.scalar.dma_start(
                        out=out_ext[:, w0 * C:(b + 1) * C],
                        in_=stg[:, :(b + 1 - w0) * C])
    nc.finalize()
    return nc


# ------------------------------------------------------- host-side oracles

def _lrelu(v):
    return np.maximum(v, 0.01 * v)


def host_h1s_table(pre):
    """Expected full h1s table [NCORES*NPAD, D] in phase-A (p,b) row order."""
    xT = pre["xT"]
    out = np.zeros((NCORES * NPAD, D), np.float32)
    for c in range(NCORES):
        xc = xT[c].T
        h = _lrelu(xc @ pre["W1"] + pre["b1row"])
        h = h @ pre["W2"] + pre["b2row"]
        h = h * pre["invsA"][c].T.reshape(-1, 1)
        hpb = h.reshape(NB, P, D).transpose(1, 0, 2).reshape(NPAD, D)
        out[c * NPAD:(c + 1) * NPAD] = hpb
    return out


def host_agg(pre, tabs, dim):
    """Segment-sum using the halo tables (matches the device data path)."""
    KS = pre["KS"]
    which = _split_of(pre["splits"])
    mask = np.asarray(pre["mask"]).astype(np.float32).reshape(
        NCORES, P, NB, KS, P)
    out = np.zeros((NCORES, P, NB, dim), np.float32)
    for c in range(NCORES):
        for s, (lo, hi) in enumerate(pre["splits"]):
            uniq = pre["gids"][c][s]
            nodes = pre["sendnode"][c, lo:hi]
            local = np.zeros(nodes.shape, np.int64)
            valid = nodes >= 0
            local[valid] = np.searchsorted(uniq, nodes[valid])
            tabf = np.asarray(tabs[c][s]).astype(np.float32)
            g = tabf[local][:, :, :dim]                     # [nb, KSLOT, dim]
            g = g.reshape(hi - lo, KS, P, dim)              # [nb, k, lane, d]
            out[c, :, lo:hi] = np.einsum(
                "lbkq,bkld->qbd", mask[c, :, lo:hi], g)
    return out


def host_h2s_table(pre, tabsB):
    agg = host_agg(pre, tabsB, D)
    out = np.zeros((NCORES * NPAD, C), np.float32)
    for c in range(NCORES):
        v = agg[c] * pre["invr_blk"][c][:, :, None]
        h = _lrelu(v)
        y = h.reshape(-1, D) @ pre["Wd"] + pre["bdrow"]
        y = y.reshape(P, NB, C) * pre["invs_blk"][c][:, :, None]
        out[c * NPAD:(c + 1) * NPAD] = y.reshape(NPAD, C)
    return out


def host_final(pre, tabsC):
    agg = host_agg(pre, tabsC, C)
    res = np.zeros((NCORES, NPAD, C), np.float32)
    for c in range(NCORES):
        v = agg[c] * pre["invr_blk"][c][:, :, None]
        e = np.exp(v)
        res[c] = (e / e.sum(-1, keepdims=True)).reshape(NPAD, C)
    return res


def unshard(pre, res_list):
    out = np.zeros((N, C), np.float32)
    for c in range(NCORES):
        r = np.asarray(res_list[c], np.float32).reshape(P, NB, C)
        nid = pre["node_at"][c]                  # [NB, P]
        valid = nid >= 0
        out[nid[valid]] = r.transpose(1, 0, 2)[valid]
    return out


# ------------------------------------------------------------------ driver

_EXEC_TIMES = []


def _run(nc, in_maps):
    from concourse.bass_utils import run_bass_kernel_spmd
    res = run_bass_kernel_spmd(nc, in_maps, core_ids=list(range(NCORES)))
    if res.exec_time_ns is not None:
        _EXEC_TIMES.append(res.exec_time_ns)
    return res.results


def kernel(x, senders, receivers, W1, b1, W2, b2, Wd, bd):
    pre = preprocess(x, senders, receivers, W1, b1, W2, b2, Wd, bd)
    KS = pre["KS"]

    nc_a = _build_phase_a(pre["bias_nz"])
    res_a = _run(nc_a, maps_a(pre))
    h1s = np.concatenate(
        [np.asarray(r["h1s"]).reshape(NPAD, D) for r in res_a], axis=0)
    tabsB = build_halo_tabs(pre, h1s, pre["rowA"], D)

    nc_b = _build_phase_b(KS, pre["splits"], pre["bias_nz"])
    res_b = _run(nc_b, maps_b(pre, tabsB))
    h2s = np.concatenate(
        [np.asarray(r["h2s"]).reshape(NPAD, C) for r in res_b], axis=0)
    tabsC = build_halo_tabs(pre, h2s, pre["rowB"], C)

    nc_c = _build_phase_c(KS, pre["splits"])
    res_c = _run(nc_c, maps_c(pre, tabsC))
    return unshard(pre, [r["res"] for r in res_c])


# revision 27
# speedup vs baseline: 1.0671x; 1.0671x over previous
"""GCN (2-layer graph convolution, symmetric norm) on 8 TRN2 NeuronCores.

Design (graph/data parallel per sharding hint, optimized for the TRN2 cost
model: per-DMA fixed costs, single SWDGE gather queue, bf16 tensor engine):

 - Host preprocessing (indices/layout only): degrees, edge sort, a
   bin-packing of receivers into 8*98 blocks of 128 slots balancing edge
   count (so every block needs exactly KS=6 gather groups of 128 edges),
   fp8 one-hot segment-sum masks, int16 local gather indices, and per-core
   halo tables (each core receives only the node rows its edges reference,
   split into block-ranges so local ids fit in int16).
 - Phase A (node-sharded): h1s = lrelu(x@W1+b1) @ W2 + b2 in feature-major
   chunks; bf16 after the first matmul; writes the bf16 h1s table in
   (partition, block) row order.
 - Phase B (edge-sharded): per receiver block, ONE dma_gather fetches
   6x128 sender rows (bf16) from the core's halo table; fp8 one-hot masks
   (streamed on the idle SP queue) segment-sum via PE matmuls;
   lrelu(agg*invr) @ Wd + bd scaled by invs -> bf16 h2s table.
 - Phase C: same aggregation over the h2s halo (rows padded to 256B for
   dma_gather), then softmax via Exp on the scalar engine (logits are O(10);
   no max-subtract needed) + DVE row-sum/reciprocal.
Host does only index preprocessing, layout permutation, and shard (halo)
assembly between phases.
"""

import numpy as np
import ml_dtypes

N = 100000
E = 600000
D = 128
C = 40
NCORES = 8
NS = N // NCORES          # 12500 nodes per core
P = 128
NB = (NS + P - 1) // P    # 98 receiver blocks per core
NPAD = NB * P             # 12544
NBINS = NCORES * NB       # 784
TROWS = 32768             # halo table rows (int16-addressable)

BF16 = ml_dtypes.bfloat16
FP8 = ml_dtypes.float8_e4m3


# ---------------------------------------------------------------- host side

def preprocess(x, senders, receivers, W1, b1, W2, b2, Wd, bd):
    x = np.asarray(x, np.float32)
    senders = np.asarray(senders, np.int64)
    receivers = np.asarray(receivers, np.int64)
    pre = {
        "W1": np.ascontiguousarray(np.asarray(W1, np.float32)),
        "W2": np.ascontiguousarray(np.asarray(W2, np.float32)),
        "Wd": np.ascontiguousarray(np.asarray(Wd, np.float32)),
        "b1row": np.asarray(b1, np.float32).reshape(1, D),
        "b2row": np.asarray(b2, np.float32).reshape(1, D),
        "bdrow": np.asarray(bd, np.float32).reshape(1, C),
    }

    deg_s = np.bincount(senders, minlength=N).astype(np.float32)
    deg_r = np.bincount(receivers, minlength=N).astype(np.float32)
    inv_s = (1.0 / np.sqrt(np.maximum(deg_s, 1.0))).astype(np.float32)
    inv_r = (1.0 / np.sqrt(np.maximum(deg_r, 1.0))).astype(np.float32)

    # --- bin-pack receivers into NBINS bins of <=128 slots, balancing edges
    import heapq
    order = np.argsort(-deg_r, kind="stable")
    heap = [(0.0, b) for b in range(NBINS)]
    heapq.heapify(heap)
    slots_used = np.zeros(NBINS, np.int32)
    assign_bin = np.empty(N, np.int32)
    slot_p = np.empty(N, np.int32)
    for n in order:
        while True:
            load, b = heapq.heappop(heap)
            if slots_used[b] < P:
                break
        assign_bin[n] = b
        slot_p[n] = slots_used[b]
        slots_used[b] += 1
        heapq.heappush(heap, (load + float(deg_r[n]), b))

    bin_load = np.bincount(assign_bin[receivers], minlength=NBINS)
    KS = int(np.ceil(bin_load.max() / P))
    pre["KS"] = KS

    # --- table-row maps
    # phase A table order: node n -> row (n//NS)*NPAD + (n%NS % P)*NB + (n%NS//P)
    nn = np.arange(N, dtype=np.int64)
    loc = nn % NS
    rowA = (nn // NS) * NPAD + (loc % P) * NB + (loc // P)
    # phase B table order: node n -> its aggregation slot row
    rowB = (assign_bin.astype(np.int64) // NB) * NPAD + \
        slot_p.astype(np.int64) * NB + (assign_bin.astype(np.int64) % NB)
    pre["rowA"] = rowA
    pre["rowB"] = rowB

    # --- edge slot assignment: per (core, block), k-major flat slot list
    ebin = assign_bin[receivers]
    eorder = np.argsort(ebin, kind="stable")
    ebin_s = ebin[eorder].astype(np.int64)
    esend = senders[eorder]
    eq = slot_p[receivers][eorder].astype(np.int64)          # local recv slot
    binstarts = np.searchsorted(ebin_s, np.arange(NBINS))
    pos = np.arange(E, dtype=np.int64) - binstarts[ebin_s]
    ek = pos // P
    elane = pos % P
    ec = ebin_s // NB
    eb = ebin_s % NB

    KSLOT = KS * P
    sendnode = np.full((NCORES, NB, KSLOT), -1, np.int64)
    sendnode[ec, eb, ek * P + elane] = esend
    pre["sendnode"] = sendnode

    mask = np.zeros((NCORES, P, NB * KS * P), np.uint8)
    mask[ec, elane, (eb * KS + ek) * P + eq] = 0x38          # fp8e4m3 1.0
    pre["mask"] = mask.view(FP8)

    # --- halo split: block ranges with <=TROWS-1 unique senders each
    nsplit = 2
    while True:
        bounds = np.linspace(0, NB, nsplit + 1).astype(int)
        splits = [(int(bounds[i]), int(bounds[i + 1])) for i in range(nsplit)]
        gids = []        # [c][s] -> node ids in halo table order
        ok = True
        for c in range(NCORES):
            row = []
            for lo, hi in splits:
                nodes = sendnode[c, lo:hi]
                uniq = np.unique(nodes[nodes >= 0])
                if uniq.shape[0] > TROWS - 1:
                    ok = False
                row.append(uniq)
            gids.append(row)
            if not ok:
                break
        if ok:
            break
        nsplit += 1
    pre["splits"] = splits
    pre["gids"] = gids

    # --- int16 packed gather indices (wrapped in 16 partitions, replicated)
    idx16 = np.zeros((NCORES, P, NB * KS * 8), np.int16)
    for c in range(NCORES):
        for s, (lo, hi) in enumerate(splits):
            uniq = gids[c][s]
            nodes = sendnode[c, lo:hi]                       # [nb, KSLOT]
            local = np.zeros(nodes.shape, np.int64)
            valid = nodes >= 0
            local[valid] = np.searchsorted(uniq, nodes[valid])
            # wrap each block's flat list: w[i, t] = flat[t*16 + i]
            nb = hi - lo
            w = local.reshape(nb, KSLOT // 16, 16).transpose(0, 2, 1)
            w = w.reshape(nb, 16, KSLOT // 16)
            idx16[c, :, lo * KS * 8:hi * KS * 8] = np.tile(
                w, (1, 8, 1)).transpose(1, 0, 2).reshape(P, nb * KS * 8)
    pre["idx16"] = idx16

    # --- per-slot scale vectors
    node_at = np.full((NCORES, NB, P), -1, np.int64)
    ab = assign_bin.astype(np.int64)
    node_at[ab // NB, ab % NB, slot_p] = nn
    pre["node_at"] = node_at
    safe = np.maximum(node_at, 0)
    invr_blk = np.where(node_at >= 0, inv_r[safe], 1.0).astype(np.float32)
    invs_blk = np.where(node_at >= 0, inv_s[safe], 1.0).astype(np.float32)
    pre["invr_blk"] = np.ascontiguousarray(invr_blk.transpose(0, 2, 1))  # [c,P,NB]
    pre["invs_blk"] = np.ascontiguousarray(invs_blk.transpose(0, 2, 1))

    # phase-A-order inv_s: [c, P, NB]
    invsA = np.ones((NCORES, NPAD), np.float32)
    invsA[:, :NS] = inv_s.reshape(NCORES, NS)
    pre["invsA"] = np.ascontiguousarray(
        invsA.reshape(NCORES, NB, P).transpose(0, 2, 1))

    # x transposed per core: [c, D, NPAD]
    xT = np.zeros((NCORES, D, NPAD), np.float32)
    xT[:, :, :NS] = x.reshape(NCORES, NS, D).transpose(0, 2, 1)
    pre["xT"] = xT

    pre["ones512"] = np.ones((1, 512), np.float32)
    pre["inv_s"] = inv_s
    pre["inv_r"] = inv_r
    pre["bias_nz"] = bool(np.any(pre["b1row"]) or np.any(pre["b2row"])
                          or np.any(pre["bdrow"]))
    return pre


def build_halo_tabs(pre, table_full, rowmap, width):
    """Per-core halo tables [nsplit][TROWS, P] bf16 from a full table."""
    tabs = []
    tf = np.asarray(table_full)
    for c in range(NCORES):
        row = []
        for s in range(len(pre["splits"])):
            gid = pre["gids"][c][s]
            t = np.zeros((TROWS, P), BF16)
            t[:gid.shape[0], :width] = tf[rowmap[gid], :width]
            row.append(t)
        tabs.append(row)
    return tabs


def maps_a(pre):
    return [
        {"xT": pre["xT"][c], "W1": pre["W1"], "W2": pre["W2"],
         "b1row": pre["b1row"], "b2row": pre["b2row"],
         "ones512": pre["ones512"], "invsA": pre["invsA"][c]}
        for c in range(NCORES)
    ]


def maps_b(pre, tabs):
    return [
        {**{f"tab{s}": tabs[c][s] for s in range(len(pre["splits"]))},
         "idx": pre["idx16"][c], "mask": pre["mask"][c],
         "invr": pre["invr_blk"][c], "invs": pre["invs_blk"][c],
         "invris": pre["invr_blk"][c] * pre["invs_blk"][c],
         "Wd": pre["Wd"], "bdrow": pre["bdrow"], "ones512": pre["ones512"]}
        for c in range(NCORES)
    ]


def maps_c(pre, tabs):
    return [
        {**{f"tab{s}": tabs[c][s] for s in range(len(pre["splits"]))},
         "idx": pre["idx16"][c], "mask": pre["mask"][c],
         "invr": pre["invr_blk"][c]}
        for c in range(NCORES)
    ]


# ------------------------------------------------------------- bass kernels

def _chunks():
    out = []
    b = 0
    while b < NB:
        w = min(4, NB - b)
        out.append((b, w))
        b += w
    return out


def _build_phase_a(bias_nz=True):
    from concourse import bacc, mybir, tile

    f32 = mybir.dt.float32
    bf16 = mybir.dt.bfloat16
    nc = bacc.Bacc("TRN2", target_bir_lowering=False, debug=False)
    xT_ext = nc.declare_dram_parameter("xT", [D, NPAD], f32, isOutput=False)
    w1_ext = nc.declare_dram_parameter("W1", [D, D], f32, isOutput=False)
    w2_ext = nc.declare_dram_parameter("W2", [D, D], f32, isOutput=False)
    b1_ext = nc.declare_dram_parameter("b1row", [1, D], f32, isOutput=False)
    b2_ext = nc.declare_dram_parameter("b2row", [1, D], f32, isOutput=False)
    ones_ext = nc.declare_dram_parameter("ones512", [1, 512], f32, isOutput=False)
    invsA_ext = nc.declare_dram_parameter("invsA", [P, NB], f32, isOutput=False)
    out_ext = nc.declare_dram_parameter("h1s", [P, NB * D], bf16, isOutput=True)

    with tile.TileContext(nc) as tc:
        with (
            tc.tile_pool(name="const", bufs=1) as cp,
            tc.tile_pool(name="xin", bufs=4) as xp,
            tc.tile_pool(name="work", bufs=4) as sp,
            tc.tile_pool(name="stg", bufs=4) as gp,
            tc.tile_pool(name="ps1", bufs=3, space="PSUM") as pp1,
            tc.tile_pool(name="ps2", bufs=3, space="PSUM") as pp2,
        ):
            w1 = cp.tile([D, D], dtype=f32)
            nc.sync.dma_start(out=w1[:], in_=w1_ext[:])
            w2f = cp.tile([D, D], dtype=f32)
            nc.sync.dma_start(out=w2f[:], in_=w2_ext[:])
            w2 = cp.tile([D, D], dtype=bf16)
            nc.vector.tensor_copy(w2[:], w2f[:])
            b1 = cp.tile([1, D], dtype=f32)
            nc.sync.dma_start(out=b1[:], in_=b1_ext[:])
            b2 = cp.tile([1, D], dtype=f32)
            nc.sync.dma_start(out=b2[:], in_=b2_ext[:])
            ones = cp.tile([1, 512], dtype=f32)
            nc.sync.dma_start(out=ones[:], in_=ones_ext[:])
            invsA = cp.tile([P, NB], dtype=f32)
            nc.sync.dma_start(out=invsA[:], in_=invsA_ext[:])

            for ci, (b0, w) in enumerate(_chunks()):
                cw = w * P
                c0 = b0 * P
                xt = xp.tile([D, cw], dtype=f32)
                ldq = nc.sync if ci % 2 == 0 else nc.gpsimd
                ldq.dma_start(out=xt[:], in_=xT_ext[:, c0:c0 + cw])
                # y1 = x@W1 (+ b1), feature-major [D, cw]
                ps1 = pp1.tile([P, cw], dtype=f32, space="PSUM")
                nc.tensor.matmul(out=ps1[:], lhsT=w1[:], rhs=xt[:],
                                 start=True, stop=not bias_nz)
                if bias_nz:
                    nc.tensor.matmul(out=ps1[:], lhsT=b1[:],
                                     rhs=ones[:, :cw], start=False, stop=True)
                # lrelu: t01 = 0.01*y1 (Act), z1 = max(y1, t01) (DVE)
                t01 = sp.tile([P, cw], dtype=bf16)
                nc.scalar.mul(t01[:], ps1[:], 0.01)
                z1 = sp.tile([P, cw], dtype=bf16)
                nc.vector.tensor_tensor(out=z1[:], in0=ps1[:], in1=t01[:],
                                        op=mybir.AluOpType.max)
                # per 128-node block: y2 = z1_blk.T @ W2 (+ b2), node-major
                stg = gp.tile([P, cw], dtype=bf16)
                for j in range(w):
                    ps2 = pp2.tile([P, D], dtype=f32, space="PSUM")
                    nc.tensor.matmul(out=ps2[:], lhsT=z1[:, j * P:(j + 1) * P],
                                     rhs=w2[:], start=True, stop=not bias_nz)
                    if bias_nz:
                        nc.tensor.matmul(out=ps2[:], lhsT=b2[:],
                                         rhs=ones[:, :D], start=False, stop=True)
                    bcol = b0 + j
                    dst = stg[:, j * P:(j + 1) * P]
                    if j % 2 == 1:
                        nc.scalar.activation(
                            out=dst, in_=ps2[:],
                            func=mybir.ActivationFunctionType.Copy,
                            bias=0.0, scale=invsA[:, bcol:bcol + 1])
                    else:
                        sv = invsA[:, bcol:bcol + 1].to_broadcast([P, D])
                        nc.vector.tensor_tensor(out=dst, in0=ps2[:], in1=sv,
                                                op=mybir.AluOpType.mult)
                wrq = nc.gpsimd if ci % 2 == 0 else nc.sync
                wrq.dma_start(out=out_ext[:, c0:c0 + cw], in_=stg[:])
    nc.finalize()
    return nc


def _split_of(splits):
    which = np.empty(NB, np.int32)
    for s, (lo, hi) in enumerate(splits):
        which[lo:hi] = s
    return which


def _build_phase_b(KS, splits, bias_nz=True):
    from concourse import bacc, mybir, tile
    from concourse.masks import make_identity

    f32 = mybir.dt.float32
    bf16 = mybir.dt.bfloat16
    fp8 = mybir.dt.float8e4
    i16 = mybir.dt.int16
    nc = bacc.Bacc("TRN2", target_bir_lowering=False, debug=False)
    tab_exts = [nc.declare_dram_parameter(f"tab{s}", [TROWS, P], bf16,
                                          isOutput=False)
                for s in range(len(splits))]
    idx_ext = nc.declare_dram_parameter("idx", [P, NB * KS * 8], i16,
                                        isOutput=False)
    mask_ext = nc.declare_dram_parameter("mask", [P, NB * KS * P], fp8,
                                         isOutput=False)
    invr_ext = nc.declare_dram_parameter("invr", [P, NB], f32, isOutput=False)
    invs_ext = nc.declare_dram_parameter("invs", [P, NB], f32, isOutput=False)
    invris_ext = nc.declare_dram_parameter("invris", [P, NB], f32, isOutput=False)
    wd_ext = nc.declare_dram_parameter("Wd", [D, C], f32, isOutput=False)
    bd_ext = nc.declare_dram_parameter("bdrow", [1, C], f32, isOutput=False)
    ones_ext = nc.declare_dram_parameter("ones512", [1, 512], f32, isOutput=False)
    out_ext = nc.declare_dram_parameter("h2s", [P, NB * C], bf16, isOutput=True)

    which = _split_of(splits)
    WGRP = 16

    with tile.TileContext(nc) as tc:
        with (
            tc.tile_pool(name="const", bufs=1) as cp,
            tc.tile_pool(name="gat", bufs=6) as gp,
            tc.tile_pool(name="msk", bufs=4) as mp,
            tc.tile_pool(name="work", bufs=6) as sp,
            tc.tile_pool(name="stg", bufs=2) as op,
            tc.tile_pool(name="psA", bufs=4, space="PSUM") as ppA,
            tc.tile_pool(name="psT", bufs=2, space="PSUM") as ppT,
            tc.tile_pool(name="psO", bufs=2, space="PSUM") as ppO,
        ):
            idx = cp.tile([P, NB * KS * 8], dtype=i16)
            nc.sync.dma_start(out=idx[:], in_=idx_ext[:])
            invr = cp.tile([P, NB], dtype=f32)
            nc.sync.dma_start(out=invr[:], in_=invr_ext[:])
            invs = cp.tile([P, NB], dtype=f32)
            nc.sync.dma_start(out=invs[:], in_=invs_ext[:])
            invris = cp.tile([P, NB], dtype=f32)
            nc.sync.dma_start(out=invris[:], in_=invris_ext[:])
            wdf = cp.tile([D, C], dtype=f32)
            nc.sync.dma_start(out=wdf[:], in_=wd_ext[:])
            wd = cp.tile([D, C], dtype=bf16)
            nc.vector.tensor_copy(wd[:], wdf[:])
            bd = cp.tile([1, C], dtype=f32)
            nc.sync.dma_start(out=bd[:], in_=bd_ext[:])
            ones = cp.tile([1, 512], dtype=f32)
            nc.sync.dma_start(out=ones[:], in_=ones_ext[:])
            identb = cp.tile([P, P], dtype=bf16)
            make_identity(nc, identb[:])

            mk = None
            stg = None
            for b in range(NB):
                if b % 2 == 0:
                    nmk = min(2, NB - b)
                    mk = mp.tile([P, nmk * KS * P], dtype=fp8)
                    nc.sync.dma_start(
                        out=mk[:],
                        in_=mask_ext[:, b * KS * P:(b + nmk) * KS * P])
                if b % WGRP == 0:
                    nw = min(WGRP, NB - b)
                    stg = op.tile([P, nw * C], dtype=bf16)
                g = gp.tile([P, KS, P], dtype=bf16)
                nc.gpsimd.dma_gather(
                    out_ap=g[:].bitcast(f32), in_ap=tab_exts[which[b]][:].bitcast(f32),
                    idxs_ap=idx[:, b * KS * 8:(b + 1) * KS * 8],
                    num_idxs=KS * P, num_idxs_reg=KS * P, elem_size=P // 2)
                psA = ppA.tile([P, D], dtype=f32, space="PSUM")
                mo = (b % 2) * KS * P
                if not bias_nz:
                    # transposed scheme: aggT = sum_k g_k^T @ m_k  [feat, recv]
                    # lrelu commutes with the positive invr scale, which is
                    # merged with invs into the final per-receiver stage scale
                    for k in range(KS):
                        nc.tensor.matmul(
                            out=psA[:], lhsT=g[:, k, :],
                            rhs=mk[:, mo + k * P:mo + (k + 1) * P],
                            start=(k == 0), stop=(k == KS - 1))
                    za = sp.tile([P, D], dtype=bf16)
                    nc.scalar.copy(za[:], psA[:])
                    zb = sp.tile([P, D], dtype=bf16)
                    nc.vector.tensor_scalar_mul(zb[:], za[:], 0.01)
                    h = sp.tile([P, D], dtype=bf16)
                    nc.vector.tensor_tensor(out=h[:], in0=za[:], in1=zb[:],
                                            op=mybir.AluOpType.max)
                    psO = ppO.tile([P, C], dtype=f32, space="PSUM")
                    nc.tensor.matmul(out=psO[:], lhsT=h[:], rhs=wd[:],
                                     start=True, stop=True)
                    dst = stg[:, (b % WGRP) * C:(b % WGRP + 1) * C]
                    if b % 2 == 0:
                        nc.vector.tensor_tensor(
                            out=dst, in0=psO[:],
                            in1=invris[:, b:b + 1].to_broadcast([P, C]),
                            op=mybir.AluOpType.mult)
                    else:
                        nc.scalar.activation(
                            out=dst, in_=psO[:],
                            func=mybir.ActivationFunctionType.Copy,
                            bias=0.0, scale=invris[:, b:b + 1])
                else:
                    for k in range(KS):
                        nc.tensor.matmul(
                            out=psA[:], lhsT=mk[:, mo + k * P:mo + (k + 1) * P],
                            rhs=g[:, k, :], start=(k == 0), stop=(k == KS - 1))
                    za = sp.tile([P, D], dtype=bf16)
                    nc.scalar.activation(out=za[:], in_=psA[:],
                                         func=mybir.ActivationFunctionType.Copy,
                                         bias=0.0, scale=invr[:, b:b + 1])
                    zb = sp.tile([P, D], dtype=bf16)
                    nc.vector.tensor_scalar_mul(zb[:], za[:], 0.01)
                    h = sp.tile([P, D], dtype=bf16)
                    nc.vector.tensor_tensor(out=h[:], in0=za[:], in1=zb[:],
                                            op=mybir.AluOpType.max)
                    psT = ppT.tile([P, D], dtype=bf16, space="PSUM")
                    nc.tensor.transpose(out=psT[:], in_=h[:], identity=identb[:])
                    hT = sp.tile([P, D], dtype=bf16)
                    if b % 2 == 0:
                        nc.scalar.copy(hT[:], psT[:])
                    else:
                        nc.vector.tensor_copy(hT[:], psT[:])
                    psO = ppO.tile([P, C], dtype=f32, space="PSUM")
                    nc.tensor.matmul(out=psO[:], lhsT=hT[:], rhs=wd[:],
                                     start=True, stop=False)
                    nc.tensor.matmul(out=psO[:], lhsT=ones[:, :D], rhs=bd[:],
                                     start=False, stop=True)
                    nc.vector.tensor_tensor(
                        out=stg[:, (b % WGRP) * C:(b % WGRP + 1) * C],
                        in0=psO[:], in1=invs[:, b:b + 1].to_broadcast([P, C]),
                        op=mybir.AluOpType.mult)
                if b % WGRP == WGRP - 1 or b == NB - 1:
                    w0 = (b // WGRP) * WGRP
                    nc.sync.dma_start(
                        out=out_ext[:, w0 * C:(b + 1) * C],
                        in_=stg[:, :(b + 1 - w0) * C])
    nc.finalize()
    return nc


def _build_phase_c(KS, splits):
    from concourse import bacc, mybir, tile

    f32 = mybir.dt.float32
    bf16 = mybir.dt.bfloat16
    fp8 = mybir.dt.float8e4
    i16 = mybir.dt.int16
    nc = bacc.Bacc("TRN2", target_bir_lowering=False, debug=False)
    tab_exts = [nc.declare_dram_parameter(f"tab{s}", [TROWS, P], bf16,
                                          isOutput=False)
                for s in range(len(splits))]
    idx_ext = nc.declare_dram_parameter("idx", [P, NB * KS * 8], i16,
                                        isOutput=False)
    mask_ext = nc.declare_dram_parameter("mask", [P, NB * KS * P], fp8,
                                         isOutput=False)
    invr_ext = nc.declare_dram_parameter("invr", [P, NB], f32, isOutput=False)
    out_ext = nc.declare_dram_parameter("res", [P, NB * C], f32, isOutput=True)

    which = _split_of(splits)
    WGRP = 16

    with tile.TileContext(nc) as tc:
        with (
            tc.tile_pool(name="const", bufs=1) as cp,
            tc.tile_pool(name="gat", bufs=6) as gp,
            tc.tile_pool(name="msk", bufs=4) as mp,
            tc.tile_pool(name="work", bufs=6) as sp,
            tc.tile_pool(name="stg", bufs=2) as op,
            tc.tile_pool(name="psC", bufs=6, space="PSUM") as ppC,
        ):
            idx = cp.tile([P, NB * KS * 8], dtype=i16)
            nc.sync.dma_start(out=idx[:], in_=idx_ext[:])
            invr = cp.tile([P, NB], dtype=f32)
            nc.sync.dma_start(out=invr[:], in_=invr_ext[:])

            mk = None
            stg = None
            for b in range(NB):
                if b % 2 == 0:
                    nmk = min(2, NB - b)
                    mk = mp.tile([P, nmk * KS * P], dtype=fp8)
                    nc.sync.dma_start(
                        out=mk[:],
                        in_=mask_ext[:, b * KS * P:(b + nmk) * KS * P])
                if b % WGRP == 0:
                    nw = min(WGRP, NB - b)
                    stg = op.tile([P, nw * C], dtype=f32)
                g = gp.tile([P, KS, P], dtype=bf16)
                nc.gpsimd.dma_gather(
                    out_ap=g[:].bitcast(f32), in_ap=tab_exts[which[b]][:].bitcast(f32),
                    idxs_ap=idx[:, b * KS * 8:(b + 1) * KS * 8],
                    num_idxs=KS * P, num_idxs_reg=KS * P, elem_size=P // 2)
                psC = ppC.tile([P, C], dtype=f32, space="PSUM")
                mo = (b % 2) * KS * P
                for k in range(KS):
                    nc.tensor.matmul(
                        out=psC[:], lhsT=mk[:, mo + k * P:mo + (k + 1) * P],
                        rhs=g[:, k, 0:C], start=(k == 0), stop=(k == KS - 1))
                # softmax: ex = exp(agg*invr) (logits O(10), no max-subtract)
                ex = sp.tile([P, C], dtype=bf16)
                nc.scalar.activation(out=ex[:], in_=psC[:],
                                     func=mybir.ActivationFunctionType.Exp,
                                     scale=invr[:, b:b + 1])
                dn = sp.tile([P, 1], dtype=f32)
                nc.vector.reduce_sum(dn[:], ex[:], axis=mybir.AxisListType.X)
                rd = sp.tile([P, 1], dtype=f32)
                nc.vector.reciprocal(rd[:], dn[:])
                nc.vector.tensor_tensor(
                    out=stg[:, (b % WGRP) * C:(b % WGRP + 1) * C],
                    in0=ex[:], in1=rd[:].to_broadcast([P, C]),
                    op=mybir.AluOpType.mult)
                if b % WGRP == WGRP - 1 or b == NB - 1:
                    w0 = (b // WGRP) * WGRP
                    nc.scalar.dma_start(
                        out=out_ext[:, w0 * C:(b + 1) * C],
                        in_=stg[:, :(b + 1 - w0) * C])
    nc.finalize()
    return nc


# ------------------------------------------------------- host-side oracles

def _lrelu(v):
    return np.maximum(v, 0.01 * v)


def host_h1s_table(pre):
    """Expected full h1s table [NCORES*NPAD, D] in phase-A (p,b) row order."""
    xT = pre["xT"]
    out = np.zeros((NCORES * NPAD, D), np.float32)
    for c in range(NCORES):
        xc = xT[c].T
        h = _lrelu(xc @ pre["W1"] + pre["b1row"])
        h = h @ pre["W2"] + pre["b2row"]
        h = h * pre["invsA"][c].T.reshape(-1, 1)
        hpb = h.reshape(NB, P, D).transpose(1, 0, 2).reshape(NPAD, D)
        out[c * NPAD:(c + 1) * NPAD] = hpb
    return out


def host_agg(pre, tabs, dim):
    """Segment-sum using the halo tables (matches the device data path)."""
    KS = pre["KS"]
    which = _split_of(pre["splits"])
    mask = np.asarray(pre["mask"]).astype(np.float32).reshape(
        NCORES, P, NB, KS, P)
    out = np.zeros((NCORES, P, NB, dim), np.float32)
    for c in range(NCORES):
        for s, (lo, hi) in enumerate(pre["splits"]):
            uniq = pre["gids"][c][s]
            nodes = pre["sendnode"][c, lo:hi]
            local = np.zeros(nodes.shape, np.int64)
            valid = nodes >= 0
            local[valid] = np.searchsorted(uniq, nodes[valid])
            tabf = np.asarray(tabs[c][s]).astype(np.float32)
            g = tabf[local][:, :, :dim]                     # [nb, KSLOT, dim]
            g = g.reshape(hi - lo, KS, P, dim)              # [nb, k, lane, d]
            out[c, :, lo:hi] = np.einsum(
                "lbkq,bkld->qbd", mask[c, :, lo:hi], g)
    return out


def host_h2s_table(pre, tabsB):
    agg = host_agg(pre, tabsB, D)
    out = np.zeros((NCORES * NPAD, C), np.float32)
    for c in range(NCORES):
        if pre["bias_nz"]:
            v = agg[c] * pre["invr_blk"][c][:, :, None]
            h = _lrelu(v)
            y = h.reshape(-1, D) @ pre["Wd"] + pre["bdrow"]
            y = y.reshape(P, NB, C) * pre["invs_blk"][c][:, :, None]
        else:
            h = _lrelu(agg[c])
            y = (h.reshape(-1, D) @ pre["Wd"]).reshape(P, NB, C)
            y = y * (pre["invr_blk"][c] * pre["invs_blk"][c])[:, :, None]
        out[c * NPAD:(c + 1) * NPAD] = y.reshape(NPAD, C)
    return out


def host_final(pre, tabsC):
    agg = host_agg(pre, tabsC, C)
    res = np.zeros((NCORES, NPAD, C), np.float32)
    for c in range(NCORES):
        v = agg[c] * pre["invr_blk"][c][:, :, None]
        e = np.exp(v)
        res[c] = (e / e.sum(-1, keepdims=True)).reshape(NPAD, C)
    return res


def unshard(pre, res_list):
    out = np.zeros((N, C), np.float32)
    for c in range(NCORES):
        r = np.asarray(res_list[c], np.float32).reshape(P, NB, C)
        nid = pre["node_at"][c]                  # [NB, P]
        valid = nid >= 0
        out[nid[valid]] = r.transpose(1, 0, 2)[valid]
    return out


# ------------------------------------------------------------------ driver

_EXEC_TIMES = []


def _run(nc, in_maps):
    from concourse.bass_utils import run_bass_kernel_spmd
    res = run_bass_kernel_spmd(nc, in_maps, core_ids=list(range(NCORES)))
    if res.exec_time_ns is not None:
        _EXEC_TIMES.append(res.exec_time_ns)
    return res.results


def kernel(x, senders, receivers, W1, b1, W2, b2, Wd, bd):
    pre = preprocess(x, senders, receivers, W1, b1, W2, b2, Wd, bd)
    KS = pre["KS"]

    nc_a = _build_phase_a(pre["bias_nz"])
    res_a = _run(nc_a, maps_a(pre))
    h1s = np.concatenate(
        [np.asarray(r["h1s"]).reshape(NPAD, D) for r in res_a], axis=0)
    tabsB = build_halo_tabs(pre, h1s, pre["rowA"], D)

    nc_b = _build_phase_b(KS, pre["splits"], pre["bias_nz"])
    res_b = _run(nc_b, maps_b(pre, tabsB))
    h2s = np.concatenate(
        [np.asarray(r["h2s"]).reshape(NPAD, C) for r in res_b], axis=0)
    tabsC = build_halo_tabs(pre, h2s, pre["rowB"], C)

    nc_c = _build_phase_c(KS, pre["splits"])
    res_c = _run(nc_c, maps_c(pre, tabsC))
    return unshard(pre, [r["res"] for r in res_c])


# revision 29
# speedup vs baseline: 1.0820x; 1.0140x over previous
"""GCN (2-layer graph convolution, symmetric norm) on 8 TRN2 NeuronCores.

Design (graph/data parallel per sharding hint, optimized for the TRN2 cost
model: per-DMA fixed costs, single SWDGE gather queue, bf16 tensor engine):

 - Host preprocessing (indices/layout only): degrees, edge sort, a
   bin-packing of receivers into 8*98 blocks of 128 slots balancing edge
   count (so every block needs exactly KS=6 gather groups of 128 edges),
   fp8 one-hot segment-sum masks, int16 local gather indices, and per-core
   halo tables (each core receives only the node rows its edges reference,
   split into block-ranges so local ids fit in int16).
 - Phase A (node-sharded): h1s = lrelu(x@W1+b1) @ W2 + b2 in feature-major
   chunks; bf16 after the first matmul; writes the bf16 h1s table in
   (partition, block) row order.
 - Phase B (edge-sharded): per receiver block, ONE dma_gather fetches
   6x128 sender rows (bf16) from the core's halo table; fp8 one-hot masks
   (streamed on the idle SP queue) segment-sum via PE matmuls in transposed
   orientation (aggT = sum_k g_k^T @ m_k), so lrelu feeds Wd directly with
   no transpose; since lrelu commutes with positive scales, invr*invs is a
   single per-receiver scale after Wd -> bf16 h2s table.
 - Phase C: same aggregation over the h2s halo (rows padded to 256B for
   dma_gather), then softmax via Exp on the scalar engine (logits are O(10);
   no max-subtract needed) + DVE row-sum/reciprocal.
Host does only index preprocessing, layout permutation, and shard (halo)
assembly between phases.
"""

import numpy as np
import ml_dtypes

N = 100000
E = 600000
D = 128
C = 40
NCORES = 8
NS = N // NCORES          # 12500 nodes per core
P = 128
NB = (NS + P - 1) // P    # 98 receiver blocks per core
NPAD = NB * P             # 12544
NBINS = NCORES * NB       # 784
TROWS = 32768             # halo table rows (int16-addressable)

BF16 = ml_dtypes.bfloat16
FP8 = ml_dtypes.float8_e4m3


# ---------------------------------------------------------------- host side

def preprocess(x, senders, receivers, W1, b1, W2, b2, Wd, bd):
    x = np.asarray(x, np.float32)
    senders = np.asarray(senders, np.int64)
    receivers = np.asarray(receivers, np.int64)
    pre = {
        "W1": np.ascontiguousarray(np.asarray(W1, np.float32)),
        "W2": np.ascontiguousarray(np.asarray(W2, np.float32)),
        "Wd": np.ascontiguousarray(np.asarray(Wd, np.float32)),
        "b1row": np.asarray(b1, np.float32).reshape(1, D),
        "b2row": np.asarray(b2, np.float32).reshape(1, D),
        "bdrow": np.asarray(bd, np.float32).reshape(1, C),
    }

    deg_s = np.bincount(senders, minlength=N).astype(np.float32)
    deg_r = np.bincount(receivers, minlength=N).astype(np.float32)
    inv_s = (1.0 / np.sqrt(np.maximum(deg_s, 1.0))).astype(np.float32)
    inv_r = (1.0 / np.sqrt(np.maximum(deg_r, 1.0))).astype(np.float32)

    # --- bin-pack receivers into NBINS bins of <=128 slots, balancing edges
    import heapq
    order = np.argsort(-deg_r, kind="stable")
    heap = [(0.0, b) for b in range(NBINS)]
    heapq.heapify(heap)
    slots_used = np.zeros(NBINS, np.int32)
    assign_bin = np.empty(N, np.int32)
    slot_p = np.empty(N, np.int32)
    for n in order:
        while True:
            load, b = heapq.heappop(heap)
            if slots_used[b] < P:
                break
        assign_bin[n] = b
        slot_p[n] = slots_used[b]
        slots_used[b] += 1
        heapq.heappush(heap, (load + float(deg_r[n]), b))

    bin_load = np.bincount(assign_bin[receivers], minlength=NBINS)
    KS = int(np.ceil(bin_load.max() / P))
    pre["KS"] = KS

    # --- table-row maps
    # phase A table order: node n -> row (n//NS)*NPAD + (n%NS % P)*NB + (n%NS//P)
    nn = np.arange(N, dtype=np.int64)
    loc = nn % NS
    rowA = (nn // NS) * NPAD + (loc % P) * NB + (loc // P)
    # phase B table order: node n -> its aggregation slot row
    rowB = (assign_bin.astype(np.int64) // NB) * NPAD + \
        slot_p.astype(np.int64) * NB + (assign_bin.astype(np.int64) % NB)
    pre["rowA"] = rowA
    pre["rowB"] = rowB

    # --- edge slot assignment: per (core, block), k-major flat slot list
    ebin = assign_bin[receivers]
    eorder = np.argsort(ebin, kind="stable")
    ebin_s = ebin[eorder].astype(np.int64)
    esend = senders[eorder]
    eq = slot_p[receivers][eorder].astype(np.int64)          # local recv slot
    binstarts = np.searchsorted(ebin_s, np.arange(NBINS))
    pos = np.arange(E, dtype=np.int64) - binstarts[ebin_s]
    ek = pos // P
    elane = pos % P
    ec = ebin_s // NB
    eb = ebin_s % NB

    KSLOT = KS * P
    sendnode = np.full((NCORES, NB, KSLOT), -1, np.int64)
    sendnode[ec, eb, ek * P + elane] = esend
    pre["sendnode"] = sendnode

    mask = np.zeros((NCORES, P, NB * KS * P), np.uint8)
    mask[ec, elane, (eb * KS + ek) * P + eq] = 0x38          # fp8e4m3 1.0
    pre["mask"] = mask.view(FP8)

    # --- halo split: block ranges with <=TROWS-1 unique senders each
    nsplit = 2
    while True:
        bounds = np.linspace(0, NB, nsplit + 1).astype(int)
        splits = [(int(bounds[i]), int(bounds[i + 1])) for i in range(nsplit)]
        gids = []        # [c][s] -> node ids in halo table order
        ok = True
        for c in range(NCORES):
            row = []
            for lo, hi in splits:
                nodes = sendnode[c, lo:hi]
                uniq = np.unique(nodes[nodes >= 0])
                if uniq.shape[0] > TROWS - 1:
                    ok = False
                row.append(uniq)
            gids.append(row)
            if not ok:
                break
        if ok:
            break
        nsplit += 1
    pre["splits"] = splits
    pre["gids"] = gids

    # --- int16 packed gather indices (wrapped in 16 partitions, replicated)
    idx16 = np.zeros((NCORES, P, NB * KS * 8), np.int16)
    for c in range(NCORES):
        for s, (lo, hi) in enumerate(splits):
            uniq = gids[c][s]
            nodes = sendnode[c, lo:hi]                       # [nb, KSLOT]
            local = np.zeros(nodes.shape, np.int64)
            valid = nodes >= 0
            local[valid] = np.searchsorted(uniq, nodes[valid])
            # wrap each block's flat list: w[i, t] = flat[t*16 + i]
            nb = hi - lo
            w = local.reshape(nb, KSLOT // 16, 16).transpose(0, 2, 1)
            w = w.reshape(nb, 16, KSLOT // 16)
            idx16[c, :, lo * KS * 8:hi * KS * 8] = np.tile(
                w, (1, 8, 1)).transpose(1, 0, 2).reshape(P, nb * KS * 8)
    pre["idx16"] = idx16

    # --- per-slot scale vectors
    node_at = np.full((NCORES, NB, P), -1, np.int64)
    ab = assign_bin.astype(np.int64)
    node_at[ab // NB, ab % NB, slot_p] = nn
    pre["node_at"] = node_at
    safe = np.maximum(node_at, 0)
    invr_blk = np.where(node_at >= 0, inv_r[safe], 1.0).astype(np.float32)
    invs_blk = np.where(node_at >= 0, inv_s[safe], 1.0).astype(np.float32)
    pre["invr_blk"] = np.ascontiguousarray(invr_blk.transpose(0, 2, 1))  # [c,P,NB]
    pre["invs_blk"] = np.ascontiguousarray(invs_blk.transpose(0, 2, 1))

    # phase-A-order inv_s: [c, P, NB]
    invsA = np.ones((NCORES, NPAD), np.float32)
    invsA[:, :NS] = inv_s.reshape(NCORES, NS)
    pre["invsA"] = np.ascontiguousarray(
        invsA.reshape(NCORES, NB, P).transpose(0, 2, 1))

    # x transposed per core: [c, D, NPAD]
    xT = np.zeros((NCORES, D, NPAD), np.float32)
    xT[:, :, :NS] = x.reshape(NCORES, NS, D).transpose(0, 2, 1)
    pre["xT"] = xT

    pre["ones512"] = np.ones((1, 512), np.float32)
    pre["inv_s"] = inv_s
    pre["inv_r"] = inv_r
    pre["bias_nz"] = bool(np.any(pre["b1row"]) or np.any(pre["b2row"])
                          or np.any(pre["bdrow"]))
    return pre


def build_halo_tabs(pre, table_full, rowmap, width):
    """Per-core halo tables [nsplit][TROWS, P] bf16 from a full table."""
    tabs = []
    tf = np.asarray(table_full)
    for c in range(NCORES):
        row = []
        for s in range(len(pre["splits"])):
            gid = pre["gids"][c][s]
            t = np.zeros((TROWS, P), BF16)
            t[:gid.shape[0], :width] = tf[rowmap[gid], :width]
            row.append(t)
        tabs.append(row)
    return tabs


def maps_a(pre):
    return [
        {"xT": pre["xT"][c], "W1": pre["W1"], "W2": pre["W2"],
         "b1row": pre["b1row"], "b2row": pre["b2row"],
         "ones512": pre["ones512"], "invsA": pre["invsA"][c]}
        for c in range(NCORES)
    ]


def maps_b(pre, tabs):
    return [
        {**{f"tab{s}": tabs[c][s] for s in range(len(pre["splits"]))},
         "idx": pre["idx16"][c], "mask": pre["mask"][c],
         "invr": pre["invr_blk"][c], "invs": pre["invs_blk"][c],
         "invris": pre["invr_blk"][c] * pre["invs_blk"][c],
         "Wd": pre["Wd"], "bdrow": pre["bdrow"], "ones512": pre["ones512"]}
        for c in range(NCORES)
    ]


def maps_c(pre, tabs):
    return [
        {**{f"tab{s}": tabs[c][s] for s in range(len(pre["splits"]))},
         "idx": pre["idx16"][c], "mask": pre["mask"][c],
         "invr": pre["invr_blk"][c]}
        for c in range(NCORES)
    ]


# ------------------------------------------------------------- bass kernels

def _chunks(width=2):
    out = []
    b = 0
    while b < NB:
        w = min(width, NB - b)
        out.append((b, w))
        b += w
    return out


def _build_phase_a(bias_nz=True):
    from concourse import bacc, mybir, tile

    f32 = mybir.dt.float32
    bf16 = mybir.dt.bfloat16
    nc = bacc.Bacc("TRN2", target_bir_lowering=False, debug=False)
    xT_ext = nc.declare_dram_parameter("xT", [D, NPAD], f32, isOutput=False)
    w1_ext = nc.declare_dram_parameter("W1", [D, D], f32, isOutput=False)
    w2_ext = nc.declare_dram_parameter("W2", [D, D], f32, isOutput=False)
    b1_ext = nc.declare_dram_parameter("b1row", [1, D], f32, isOutput=False)
    b2_ext = nc.declare_dram_parameter("b2row", [1, D], f32, isOutput=False)
    ones_ext = nc.declare_dram_parameter("ones512", [1, 512], f32, isOutput=False)
    invsA_ext = nc.declare_dram_parameter("invsA", [P, NB], f32, isOutput=False)
    out_ext = nc.declare_dram_parameter("h1s", [P, NB * D], bf16, isOutput=True)

    with tile.TileContext(nc) as tc:
        with (
            tc.tile_pool(name="const", bufs=1) as cp,
            tc.tile_pool(name="xin", bufs=4) as xp,
            tc.tile_pool(name="work", bufs=4) as sp,
            tc.tile_pool(name="stg", bufs=4) as gp,
            tc.tile_pool(name="ps1", bufs=3, space="PSUM") as pp1,
            tc.tile_pool(name="ps2", bufs=3, space="PSUM") as pp2,
        ):
            w1 = cp.tile([D, D], dtype=f32)
            nc.sync.dma_start(out=w1[:], in_=w1_ext[:])
            w2f = cp.tile([D, D], dtype=f32)
            nc.sync.dma_start(out=w2f[:], in_=w2_ext[:])
            w2 = cp.tile([D, D], dtype=bf16)
            nc.vector.tensor_copy(w2[:], w2f[:])
            b1 = cp.tile([1, D], dtype=f32)
            nc.sync.dma_start(out=b1[:], in_=b1_ext[:])
            b2 = cp.tile([1, D], dtype=f32)
            nc.sync.dma_start(out=b2[:], in_=b2_ext[:])
            ones = cp.tile([1, 512], dtype=f32)
            nc.sync.dma_start(out=ones[:], in_=ones_ext[:])
            invsA = cp.tile([P, NB], dtype=f32)
            nc.sync.dma_start(out=invsA[:], in_=invsA_ext[:])

            for ci, (b0, w) in enumerate(_chunks()):
                cw = w * P
                c0 = b0 * P
                xt = xp.tile([D, cw], dtype=f32)
                ldq = nc.sync if ci % 2 == 0 else nc.gpsimd
                ldq.dma_start(out=xt[:], in_=xT_ext[:, c0:c0 + cw])
                # y1 = x@W1 (+ b1), feature-major [D, cw]
                ps1 = pp1.tile([P, cw], dtype=f32, space="PSUM")
                nc.tensor.matmul(out=ps1[:], lhsT=w1[:], rhs=xt[:],
                                 start=True, stop=not bias_nz)
                if bias_nz:
                    nc.tensor.matmul(out=ps1[:], lhsT=b1[:],
                                     rhs=ones[:, :cw], start=False, stop=True)
                # lrelu: t01 = 0.01*y1 (Act), z1 = max(y1, t01) (DVE)
                t01 = sp.tile([P, cw], dtype=bf16)
                nc.scalar.mul(t01[:], ps1[:], 0.01)
                z1 = sp.tile([P, cw], dtype=bf16)
                nc.vector.tensor_tensor(out=z1[:], in0=ps1[:], in1=t01[:],
                                        op=mybir.AluOpType.max)
                # per 128-node block: y2 = z1_blk.T @ W2 (+ b2), node-major
                stg = gp.tile([P, cw], dtype=bf16)
                for j in range(w):
                    ps2 = pp2.tile([P, D], dtype=f32, space="PSUM")
                    nc.tensor.matmul(out=ps2[:], lhsT=z1[:, j * P:(j + 1) * P],
                                     rhs=w2[:], start=True, stop=not bias_nz)
                    if bias_nz:
                        nc.tensor.matmul(out=ps2[:], lhsT=b2[:],
                                         rhs=ones[:, :D], start=False, stop=True)
                    bcol = b0 + j
                    dst = stg[:, j * P:(j + 1) * P]
                    if j % 2 == 1:
                        nc.scalar.activation(
                            out=dst, in_=ps2[:],
                            func=mybir.ActivationFunctionType.Copy,
                            bias=0.0, scale=invsA[:, bcol:bcol + 1])
                    else:
                        sv = invsA[:, bcol:bcol + 1].to_broadcast([P, D])
                        nc.vector.tensor_tensor(out=dst, in0=ps2[:], in1=sv,
                                                op=mybir.AluOpType.mult)
                wrq = nc.gpsimd if ci % 2 == 0 else nc.sync
                wrq.dma_start(out=out_ext[:, c0:c0 + cw], in_=stg[:])
    nc.finalize()
    return nc


def _split_of(splits):
    which = np.empty(NB, np.int32)
    for s, (lo, hi) in enumerate(splits):
        which[lo:hi] = s
    return which


def _build_phase_b(KS, splits, bias_nz=True):
    from concourse import bacc, mybir, tile
    from concourse.masks import make_identity

    f32 = mybir.dt.float32
    bf16 = mybir.dt.bfloat16
    fp8 = mybir.dt.float8e4
    i16 = mybir.dt.int16
    nc = bacc.Bacc("TRN2", target_bir_lowering=False, debug=False)
    tab_exts = [nc.declare_dram_parameter(f"tab{s}", [TROWS, P], bf16,
                                          isOutput=False)
                for s in range(len(splits))]
    idx_ext = nc.declare_dram_parameter("idx", [P, NB * KS * 8], i16,
                                        isOutput=False)
    mask_ext = nc.declare_dram_parameter("mask", [P, NB * KS * P], fp8,
                                         isOutput=False)
    invr_ext = nc.declare_dram_parameter("invr", [P, NB], f32, isOutput=False)
    invs_ext = nc.declare_dram_parameter("invs", [P, NB], f32, isOutput=False)
    invris_ext = nc.declare_dram_parameter("invris", [P, NB], f32, isOutput=False)
    wd_ext = nc.declare_dram_parameter("Wd", [D, C], f32, isOutput=False)
    bd_ext = nc.declare_dram_parameter("bdrow", [1, C], f32, isOutput=False)
    ones_ext = nc.declare_dram_parameter("ones512", [1, 512], f32, isOutput=False)
    out_ext = nc.declare_dram_parameter("h2s", [P, NB * C], bf16, isOutput=True)

    which = _split_of(splits)
    WGRP = 16

    with tile.TileContext(nc) as tc:
        with (
            tc.tile_pool(name="const", bufs=1) as cp,
            tc.tile_pool(name="gat", bufs=6) as gp,
            tc.tile_pool(name="msk", bufs=4) as mp,
            tc.tile_pool(name="work", bufs=6) as sp,
            tc.tile_pool(name="stg", bufs=2) as op,
            tc.tile_pool(name="psA", bufs=4, space="PSUM") as ppA,
            tc.tile_pool(name="psT", bufs=2, space="PSUM") as ppT,
            tc.tile_pool(name="psO", bufs=2, space="PSUM") as ppO,
        ):
            idx = cp.tile([P, NB * KS * 8], dtype=i16)
            nc.sync.dma_start(out=idx[:], in_=idx_ext[:])
            invr = cp.tile([P, NB], dtype=f32)
            nc.sync.dma_start(out=invr[:], in_=invr_ext[:])
            invs = cp.tile([P, NB], dtype=f32)
            nc.sync.dma_start(out=invs[:], in_=invs_ext[:])
            invris = cp.tile([P, NB], dtype=f32)
            nc.sync.dma_start(out=invris[:], in_=invris_ext[:])
            wdf = cp.tile([D, C], dtype=f32)
            nc.sync.dma_start(out=wdf[:], in_=wd_ext[:])
            wd = cp.tile([D, C], dtype=bf16)
            nc.vector.tensor_copy(wd[:], wdf[:])
            bd = cp.tile([1, C], dtype=f32)
            nc.sync.dma_start(out=bd[:], in_=bd_ext[:])
            ones = cp.tile([1, 512], dtype=f32)
            nc.sync.dma_start(out=ones[:], in_=ones_ext[:])
            identb = cp.tile([P, P], dtype=bf16)
            make_identity(nc, identb[:])

            mk = None
            stg = None
            for b in range(NB):
                if b % 2 == 0:
                    nmk = min(2, NB - b)
                    mk = mp.tile([P, nmk * KS * P], dtype=fp8)
                    nc.sync.dma_start(
                        out=mk[:],
                        in_=mask_ext[:, b * KS * P:(b + nmk) * KS * P])
                if b % WGRP == 0:
                    nw = min(WGRP, NB - b)
                    stg = op.tile([P, nw * C], dtype=bf16)
                g = gp.tile([P, KS, P], dtype=bf16)
                nc.gpsimd.dma_gather(
                    out_ap=g[:].bitcast(f32), in_ap=tab_exts[which[b]][:].bitcast(f32),
                    idxs_ap=idx[:, b * KS * 8:(b + 1) * KS * 8],
                    num_idxs=KS * P, num_idxs_reg=KS * P, elem_size=P // 2)
                psA = ppA.tile([P, D], dtype=f32, space="PSUM")
                mo = (b % 2) * KS * P
                if not bias_nz:
                    # transposed scheme: aggT = sum_k g_k^T @ m_k  [feat, recv]
                    # lrelu commutes with the positive invr scale, which is
                    # merged with invs into the final per-receiver stage scale
                    for k in range(KS):
                        nc.tensor.matmul(
                            out=psA[:], lhsT=g[:, k, :],
                            rhs=mk[:, mo + k * P:mo + (k + 1) * P],
                            start=(k == 0), stop=(k == KS - 1))
                    za = sp.tile([P, D], dtype=bf16)
                    nc.scalar.copy(za[:], psA[:])
                    zb = sp.tile([P, D], dtype=bf16)
                    nc.vector.tensor_scalar_mul(zb[:], za[:], 0.01)
                    h = sp.tile([P, D], dtype=bf16)
                    nc.vector.tensor_tensor(out=h[:], in0=za[:], in1=zb[:],
                                            op=mybir.AluOpType.max)
                    psO = ppO.tile([P, C], dtype=f32, space="PSUM")
                    nc.tensor.matmul(out=psO[:], lhsT=h[:], rhs=wd[:],
                                     start=True, stop=True)
                    dst = stg[:, (b % WGRP) * C:(b % WGRP + 1) * C]
                    nc.vector.tensor_tensor(
                        out=dst, in0=psO[:],
                        in1=invris[:, b:b + 1].to_broadcast([P, C]),
                        op=mybir.AluOpType.mult)
                else:
                    for k in range(KS):
                        nc.tensor.matmul(
                            out=psA[:], lhsT=mk[:, mo + k * P:mo + (k + 1) * P],
                            rhs=g[:, k, :], start=(k == 0), stop=(k == KS - 1))
                    za = sp.tile([P, D], dtype=bf16)
                    nc.scalar.activation(out=za[:], in_=psA[:],
                                         func=mybir.ActivationFunctionType.Copy,
                                         bias=0.0, scale=invr[:, b:b + 1])
                    zb = sp.tile([P, D], dtype=bf16)
                    nc.vector.tensor_scalar_mul(zb[:], za[:], 0.01)
                    h = sp.tile([P, D], dtype=bf16)
                    nc.vector.tensor_tensor(out=h[:], in0=za[:], in1=zb[:],
                                            op=mybir.AluOpType.max)
                    psT = ppT.tile([P, D], dtype=bf16, space="PSUM")
                    nc.tensor.transpose(out=psT[:], in_=h[:], identity=identb[:])
                    hT = sp.tile([P, D], dtype=bf16)
                    if b % 2 == 0:
                        nc.scalar.copy(hT[:], psT[:])
                    else:
                        nc.vector.tensor_copy(hT[:], psT[:])
                    psO = ppO.tile([P, C], dtype=f32, space="PSUM")
                    nc.tensor.matmul(out=psO[:], lhsT=hT[:], rhs=wd[:],
                                     start=True, stop=False)
                    nc.tensor.matmul(out=psO[:], lhsT=ones[:, :D], rhs=bd[:],
                                     start=False, stop=True)
                    nc.vector.tensor_tensor(
                        out=stg[:, (b % WGRP) * C:(b % WGRP + 1) * C],
                        in0=psO[:], in1=invs[:, b:b + 1].to_broadcast([P, C]),
                        op=mybir.AluOpType.mult)
                if b % WGRP == WGRP - 1 or b == NB - 1:
                    w0 = (b // WGRP) * WGRP
                    nc.scalar.dma_start(
                        out=out_ext[:, w0 * C:(b + 1) * C],
                        in_=stg[:, :(b + 1 - w0) * C])
    nc.finalize()
    return nc


def _build_phase_c(KS, splits):
    from concourse import bacc, mybir, tile

    f32 = mybir.dt.float32
    bf16 = mybir.dt.bfloat16
    fp8 = mybir.dt.float8e4
    i16 = mybir.dt.int16
    nc = bacc.Bacc("TRN2", target_bir_lowering=False, debug=False)
    tab_exts = [nc.declare_dram_parameter(f"tab{s}", [TROWS, P], bf16,
                                          isOutput=False)
                for s in range(len(splits))]
    idx_ext = nc.declare_dram_parameter("idx", [P, NB * KS * 8], i16,
                                        isOutput=False)
    mask_ext = nc.declare_dram_parameter("mask", [P, NB * KS * P], fp8,
                                         isOutput=False)
    invr_ext = nc.declare_dram_parameter("invr", [P, NB], f32, isOutput=False)
    out_ext = nc.declare_dram_parameter("res", [P, NB * C], f32, isOutput=True)

    which = _split_of(splits)
    WGRP = 16

    with tile.TileContext(nc) as tc:
        with (
            tc.tile_pool(name="const", bufs=1) as cp,
            tc.tile_pool(name="gat", bufs=6) as gp,
            tc.tile_pool(name="msk", bufs=4) as mp,
            tc.tile_pool(name="work", bufs=6) as sp,
            tc.tile_pool(name="stg", bufs=2) as op,
            tc.tile_pool(name="psC", bufs=6, space="PSUM") as ppC,
        ):
            idx = cp.tile([P, NB * KS * 8], dtype=i16)
            nc.sync.dma_start(out=idx[:], in_=idx_ext[:])
            invr = cp.tile([P, NB], dtype=f32)
            nc.sync.dma_start(out=invr[:], in_=invr_ext[:])

            mk = None
            stg = None
            for b in range(NB):
                if b % 2 == 0:
                    nmk = min(2, NB - b)
                    mk = mp.tile([P, nmk * KS * P], dtype=fp8)
                    nc.sync.dma_start(
                        out=mk[:],
                        in_=mask_ext[:, b * KS * P:(b + nmk) * KS * P])
                if b % WGRP == 0:
                    nw = min(WGRP, NB - b)
                    stg = op.tile([P, nw * C], dtype=f32)
                g = gp.tile([P, KS, P], dtype=bf16)
                nc.gpsimd.dma_gather(
                    out_ap=g[:].bitcast(f32), in_ap=tab_exts[which[b]][:].bitcast(f32),
                    idxs_ap=idx[:, b * KS * 8:(b + 1) * KS * 8],
                    num_idxs=KS * P, num_idxs_reg=KS * P, elem_size=P // 2)
                psC = ppC.tile([P, C], dtype=f32, space="PSUM")
                mo = (b % 2) * KS * P
                for k in range(KS):
                    nc.tensor.matmul(
                        out=psC[:], lhsT=mk[:, mo + k * P:mo + (k + 1) * P],
                        rhs=g[:, k, 0:C], start=(k == 0), stop=(k == KS - 1))
                # softmax: ex = exp(agg*invr) (logits O(10), no max-subtract)
                ex = sp.tile([P, C], dtype=bf16)
                nc.scalar.activation(out=ex[:], in_=psC[:],
                                     func=mybir.ActivationFunctionType.Exp,
                                     scale=invr[:, b:b + 1])
                dn = sp.tile([P, 1], dtype=f32)
                nc.vector.reduce_sum(dn[:], ex[:], axis=mybir.AxisListType.X)
                rd = sp.tile([P, 1], dtype=f32)
                nc.vector.reciprocal(rd[:], dn[:])
                nc.vector.tensor_tensor(
                    out=stg[:, (b % WGRP) * C:(b % WGRP + 1) * C],
                    in0=ex[:], in1=rd[:].to_broadcast([P, C]),
                    op=mybir.AluOpType.mult)
                if b % WGRP == WGRP - 1 or b == NB - 1:
                    w0 = (b // WGRP) * WGRP
                    nc.scalar.dma_start(
                        out=out_ext[:, w0 * C:(b + 1) * C],
                        in_=stg[:, :(b + 1 - w0) * C])
    nc.finalize()
    return nc


# ------------------------------------------------------- host-side oracles

def _lrelu(v):
    return np.maximum(v, 0.01 * v)


def host_h1s_table(pre):
    """Expected full h1s table [NCORES*NPAD, D] in phase-A (p,b) row order."""
    xT = pre["xT"]
    out = np.zeros((NCORES * NPAD, D), np.float32)
    for c in range(NCORES):
        xc = xT[c].T
        h = _lrelu(xc @ pre["W1"] + pre["b1row"])
        h = h @ pre["W2"] + pre["b2row"]
        h = h * pre["invsA"][c].T.reshape(-1, 1)
        hpb = h.reshape(NB, P, D).transpose(1, 0, 2).reshape(NPAD, D)
        out[c * NPAD:(c + 1) * NPAD] = hpb
    return out


def host_agg(pre, tabs, dim):
    """Segment-sum using the halo tables (matches the device data path)."""
    KS = pre["KS"]
    which = _split_of(pre["splits"])
    mask = np.asarray(pre["mask"]).astype(np.float32).reshape(
        NCORES, P, NB, KS, P)
    out = np.zeros((NCORES, P, NB, dim), np.float32)
    for c in range(NCORES):
        for s, (lo, hi) in enumerate(pre["splits"]):
            uniq = pre["gids"][c][s]
            nodes = pre["sendnode"][c, lo:hi]
            local = np.zeros(nodes.shape, np.int64)
            valid = nodes >= 0
            local[valid] = np.searchsorted(uniq, nodes[valid])
            tabf = np.asarray(tabs[c][s]).astype(np.float32)
            g = tabf[local][:, :, :dim]                     # [nb, KSLOT, dim]
            g = g.reshape(hi - lo, KS, P, dim)              # [nb, k, lane, d]
            out[c, :, lo:hi] = np.einsum(
                "lbkq,bkld->qbd", mask[c, :, lo:hi], g)
    return out


def host_h2s_table(pre, tabsB):
    agg = host_agg(pre, tabsB, D)
    out = np.zeros((NCORES * NPAD, C), np.float32)
    for c in range(NCORES):
        if pre["bias_nz"]:
            v = agg[c] * pre["invr_blk"][c][:, :, None]
            h = _lrelu(v)
            y = h.reshape(-1, D) @ pre["Wd"] + pre["bdrow"]
            y = y.reshape(P, NB, C) * pre["invs_blk"][c][:, :, None]
        else:
            h = _lrelu(agg[c])
            y = (h.reshape(-1, D) @ pre["Wd"]).reshape(P, NB, C)
            y = y * (pre["invr_blk"][c] * pre["invs_blk"][c])[:, :, None]
        out[c * NPAD:(c + 1) * NPAD] = y.reshape(NPAD, C)
    return out


def host_final(pre, tabsC):
    agg = host_agg(pre, tabsC, C)
    res = np.zeros((NCORES, NPAD, C), np.float32)
    for c in range(NCORES):
        v = agg[c] * pre["invr_blk"][c][:, :, None]
        e = np.exp(v)
        res[c] = (e / e.sum(-1, keepdims=True)).reshape(NPAD, C)
    return res


def unshard(pre, res_list):
    out = np.zeros((N, C), np.float32)
    for c in range(NCORES):
        r = np.asarray(res_list[c], np.float32).reshape(P, NB, C)
        nid = pre["node_at"][c]                  # [NB, P]
        valid = nid >= 0
        out[nid[valid]] = r.transpose(1, 0, 2)[valid]
    return out


# ------------------------------------------------------------------ driver

_EXEC_TIMES = []


def _run(nc, in_maps):
    from concourse.bass_utils import run_bass_kernel_spmd
    res = run_bass_kernel_spmd(nc, in_maps, core_ids=list(range(NCORES)))
    if res.exec_time_ns is not None:
        _EXEC_TIMES.append(res.exec_time_ns)
    return res.results


def kernel(x, senders, receivers, W1, b1, W2, b2, Wd, bd):
    pre = preprocess(x, senders, receivers, W1, b1, W2, b2, Wd, bd)
    KS = pre["KS"]

    nc_a = _build_phase_a(pre["bias_nz"])
    res_a = _run(nc_a, maps_a(pre))
    h1s = np.concatenate(
        [np.asarray(r["h1s"]).reshape(NPAD, D) for r in res_a], axis=0)
    tabsB = build_halo_tabs(pre, h1s, pre["rowA"], D)

    nc_b = _build_phase_b(KS, pre["splits"], pre["bias_nz"])
    res_b = _run(nc_b, maps_b(pre, tabsB))
    h2s = np.concatenate(
        [np.asarray(r["h2s"]).reshape(NPAD, C) for r in res_b], axis=0)
    tabsC = build_halo_tabs(pre, h2s, pre["rowB"], C)

    nc_c = _build_phase_c(KS, pre["splits"])
    res_c = _run(nc_c, maps_c(pre, tabsC))
    return unshard(pre, [r["res"] for r in res_c])


# revision 31
# speedup vs baseline: 1.0928x; 1.0099x over previous
"""GCN (2-layer graph convolution, symmetric norm) on 8 TRN2 NeuronCores.

Design (graph/data parallel per sharding hint, optimized for the TRN2 cost
model: per-DMA fixed costs, single SWDGE gather queue, bf16 tensor engine):

 - Host preprocessing (indices/layout only): degrees, edge sort, a
   bin-packing of receivers into 8*98 blocks of 128 slots balancing edge
   count (so every block needs exactly KS=6 gather groups of 128 edges),
   fp8 one-hot segment-sum masks, int16 local gather indices, and per-core
   halo tables (each core receives only the node rows its edges reference,
   split into block-ranges so local ids fit in int16).
 - Phase A (node-sharded): h1s = lrelu(x@W1+b1) @ W2 + b2 in feature-major
   chunks; bf16 after the first matmul; writes the bf16 h1s table in
   (partition, block) row order.
 - Phase B (edge-sharded): per receiver block, ONE dma_gather fetches
   6x128 sender rows (bf16) from the core's halo table; fp8 one-hot masks
   (streamed on the idle SP queue) segment-sum via PE matmuls in transposed
   orientation (aggT = sum_k g_k^T @ m_k), so lrelu feeds Wd directly with
   no transpose; since lrelu commutes with positive scales, invr*invs is a
   single per-receiver scale after Wd -> bf16 h2s table.
 - Phase C: same aggregation over the h2s halo (rows padded to 256B for
   dma_gather), then softmax via Exp on the scalar engine (logits are O(10);
   no max-subtract needed) + DVE row-sum/reciprocal.
Host does only index preprocessing, layout permutation, and shard (halo)
assembly between phases.
"""

import numpy as np
import ml_dtypes

N = 100000
E = 600000
D = 128
C = 40
NCORES = 8
NS = N // NCORES          # 12500 nodes per core
P = 128
NB = (NS + P - 1) // P    # 98 receiver blocks per core
NPAD = NB * P             # 12544
NBINS = NCORES * NB       # 784
TROWS = 32768             # halo table rows (int16-addressable)

BF16 = ml_dtypes.bfloat16
FP8 = ml_dtypes.float8_e4m3


# ---------------------------------------------------------------- host side

def preprocess(x, senders, receivers, W1, b1, W2, b2, Wd, bd):
    x = np.asarray(x, np.float32)
    senders = np.asarray(senders, np.int64)
    receivers = np.asarray(receivers, np.int64)
    pre = {
        "W1": np.ascontiguousarray(np.asarray(W1, np.float32)),
        "W2": np.ascontiguousarray(np.asarray(W2, np.float32)),
        "Wd": np.ascontiguousarray(np.asarray(Wd, np.float32)),
        "b1row": np.asarray(b1, np.float32).reshape(1, D),
        "b2row": np.asarray(b2, np.float32).reshape(1, D),
        "bdrow": np.asarray(bd, np.float32).reshape(1, C),
    }

    deg_s = np.bincount(senders, minlength=N).astype(np.float32)
    deg_r = np.bincount(receivers, minlength=N).astype(np.float32)
    inv_s = (1.0 / np.sqrt(np.maximum(deg_s, 1.0))).astype(np.float32)
    inv_r = (1.0 / np.sqrt(np.maximum(deg_r, 1.0))).astype(np.float32)

    # --- bin-pack receivers into NBINS bins of <=128 slots, balancing edges
    import heapq
    order = np.argsort(-deg_r, kind="stable")
    heap = [(0.0, b) for b in range(NBINS)]
    heapq.heapify(heap)
    slots_used = np.zeros(NBINS, np.int32)
    assign_bin = np.empty(N, np.int32)
    slot_p = np.empty(N, np.int32)
    for n in order:
        while True:
            load, b = heapq.heappop(heap)
            if slots_used[b] < P:
                break
        assign_bin[n] = b
        slot_p[n] = slots_used[b]
        slots_used[b] += 1
        heapq.heappush(heap, (load + float(deg_r[n]), b))

    bin_load = np.bincount(assign_bin[receivers], minlength=NBINS)
    KS = int(np.ceil(bin_load.max() / P))
    pre["KS"] = KS

    # --- table-row maps
    # phase A table order: node n -> row (n//NS)*NPAD + (n%NS % P)*NB + (n%NS//P)
    nn = np.arange(N, dtype=np.int64)
    loc = nn % NS
    rowA = (nn // NS) * NPAD + (loc % P) * NB + (loc // P)
    # phase B table order: node n -> its aggregation slot row
    rowB = (assign_bin.astype(np.int64) // NB) * NPAD + \
        slot_p.astype(np.int64) * NB + (assign_bin.astype(np.int64) % NB)
    pre["rowA"] = rowA
    pre["rowB"] = rowB

    # --- edge slot assignment: per (core, block), k-major flat slot list
    ebin = assign_bin[receivers]
    eorder = np.argsort(ebin, kind="stable")
    ebin_s = ebin[eorder].astype(np.int64)
    esend = senders[eorder]
    eq = slot_p[receivers][eorder].astype(np.int64)          # local recv slot
    binstarts = np.searchsorted(ebin_s, np.arange(NBINS))
    pos = np.arange(E, dtype=np.int64) - binstarts[ebin_s]
    ek = pos // P
    elane = pos % P
    ec = ebin_s // NB
    eb = ebin_s % NB

    KSLOT = KS * P
    sendnode = np.full((NCORES, NB, KSLOT), -1, np.int64)
    sendnode[ec, eb, ek * P + elane] = esend
    pre["sendnode"] = sendnode

    mask = np.zeros((NCORES, P, NB * KS * P), np.uint8)
    mask[ec, elane, (eb * KS + ek) * P + eq] = 0x38          # fp8e4m3 1.0
    pre["mask"] = mask.view(FP8)

    # --- halo split: block ranges with <=TROWS-1 unique senders each
    nsplit = 2
    while True:
        bounds = np.linspace(0, NB, nsplit + 1).astype(int)
        splits = [(int(bounds[i]), int(bounds[i + 1])) for i in range(nsplit)]
        gids = []        # [c][s] -> node ids in halo table order
        ok = True
        for c in range(NCORES):
            row = []
            for lo, hi in splits:
                nodes = sendnode[c, lo:hi]
                uniq = np.unique(nodes[nodes >= 0])
                if uniq.shape[0] > TROWS - 1:
                    ok = False
                row.append(uniq)
            gids.append(row)
            if not ok:
                break
        if ok:
            break
        nsplit += 1
    pre["splits"] = splits
    pre["gids"] = gids

    # --- int16 packed gather indices (wrapped in 16 partitions, replicated)
    idx16 = np.zeros((NCORES, P, NB * KS * 8), np.int16)
    for c in range(NCORES):
        for s, (lo, hi) in enumerate(splits):
            uniq = gids[c][s]
            nodes = sendnode[c, lo:hi]                       # [nb, KSLOT]
            local = np.zeros(nodes.shape, np.int64)
            valid = nodes >= 0
            local[valid] = np.searchsorted(uniq, nodes[valid])
            # wrap each block's flat list: w[i, t] = flat[t*16 + i]
            nb = hi - lo
            w = local.reshape(nb, KSLOT // 16, 16).transpose(0, 2, 1)
            w = w.reshape(nb, 16, KSLOT // 16)
            idx16[c, :, lo * KS * 8:hi * KS * 8] = np.tile(
                w, (1, 8, 1)).transpose(1, 0, 2).reshape(P, nb * KS * 8)
    pre["idx16"] = idx16

    # --- per-slot scale vectors
    node_at = np.full((NCORES, NB, P), -1, np.int64)
    ab = assign_bin.astype(np.int64)
    node_at[ab // NB, ab % NB, slot_p] = nn
    pre["node_at"] = node_at
    safe = np.maximum(node_at, 0)
    invr_blk = np.where(node_at >= 0, inv_r[safe], 1.0).astype(np.float32)
    invs_blk = np.where(node_at >= 0, inv_s[safe], 1.0).astype(np.float32)
    pre["invr_blk"] = np.ascontiguousarray(invr_blk.transpose(0, 2, 1))  # [c,P,NB]
    pre["invs_blk"] = np.ascontiguousarray(invs_blk.transpose(0, 2, 1))

    # phase-A-order inv_s: [c, P, NB]
    invsA = np.ones((NCORES, NPAD), np.float32)
    invsA[:, :NS] = inv_s.reshape(NCORES, NS)
    pre["invsA"] = np.ascontiguousarray(
        invsA.reshape(NCORES, NB, P).transpose(0, 2, 1))

    # x transposed per core: [c, D, NPAD]
    xT = np.zeros((NCORES, D, NPAD), np.float32)
    xT[:, :, :NS] = x.reshape(NCORES, NS, D).transpose(0, 2, 1)
    pre["xT"] = xT

    pre["ones512"] = np.ones((1, 512), np.float32)
    pre["inv_s"] = inv_s
    pre["inv_r"] = inv_r
    pre["bias_nz"] = bool(np.any(pre["b1row"]) or np.any(pre["b2row"])
                          or np.any(pre["bdrow"]))
    return pre


def build_halo_tabs(pre, table_full, rowmap, width):
    """Per-core halo tables [nsplit][TROWS, P] bf16 from a full table."""
    tabs = []
    tf = np.asarray(table_full)
    for c in range(NCORES):
        row = []
        for s in range(len(pre["splits"])):
            gid = pre["gids"][c][s]
            t = np.zeros((TROWS, P), BF16)
            t[:gid.shape[0], :width] = tf[rowmap[gid], :width]
            row.append(t)
        tabs.append(row)
    return tabs


def maps_a(pre):
    return [
        {"xT": pre["xT"][c], "W1": pre["W1"], "W2": pre["W2"],
         "b1row": pre["b1row"], "b2row": pre["b2row"],
         "ones512": pre["ones512"], "invsA": pre["invsA"][c]}
        for c in range(NCORES)
    ]


def maps_b(pre, tabs):
    return [
        {**{f"tab{s}": tabs[c][s] for s in range(len(pre["splits"]))},
         "idx": pre["idx16"][c], "mask": pre["mask"][c],
         "invr": pre["invr_blk"][c], "invs": pre["invs_blk"][c],
         "invris": pre["invr_blk"][c] * pre["invs_blk"][c],
         "Wd": pre["Wd"], "bdrow": pre["bdrow"], "ones512": pre["ones512"]}
        for c in range(NCORES)
    ]


def maps_c(pre, tabs):
    return [
        {**{f"tab{s}": tabs[c][s] for s in range(len(pre["splits"]))},
         "idx": pre["idx16"][c], "mask": pre["mask"][c],
         "invr": pre["invr_blk"][c]}
        for c in range(NCORES)
    ]


# ------------------------------------------------------------- bass kernels

def _chunks(width=2):
    out = []
    b = 0
    while b < NB:
        w = min(width, NB - b)
        out.append((b, w))
        b += w
    return out


def _build_phase_a(bias_nz=True):
    from concourse import bacc, mybir, tile

    f32 = mybir.dt.float32
    bf16 = mybir.dt.bfloat16
    nc = bacc.Bacc("TRN2", target_bir_lowering=False, debug=False)
    xT_ext = nc.declare_dram_parameter("xT", [D, NPAD], f32, isOutput=False)
    w1_ext = nc.declare_dram_parameter("W1", [D, D], f32, isOutput=False)
    w2_ext = nc.declare_dram_parameter("W2", [D, D], f32, isOutput=False)
    b1_ext = nc.declare_dram_parameter("b1row", [1, D], f32, isOutput=False)
    b2_ext = nc.declare_dram_parameter("b2row", [1, D], f32, isOutput=False)
    ones_ext = nc.declare_dram_parameter("ones512", [1, 512], f32, isOutput=False)
    invsA_ext = nc.declare_dram_parameter("invsA", [P, NB], f32, isOutput=False)
    out_ext = nc.declare_dram_parameter("h1s", [P, NB * D], bf16, isOutput=True)

    with tile.TileContext(nc) as tc:
        with (
            tc.tile_pool(name="const", bufs=1) as cp,
            tc.tile_pool(name="xin", bufs=6) as xp,
            tc.tile_pool(name="work", bufs=4) as sp,
            tc.tile_pool(name="stg", bufs=6) as gp,
            tc.tile_pool(name="ps1", bufs=4, space="PSUM") as pp1,
            tc.tile_pool(name="ps2", bufs=4, space="PSUM") as pp2,
        ):
            w1 = cp.tile([D, D], dtype=f32)
            nc.sync.dma_start(out=w1[:], in_=w1_ext[:])
            w2f = cp.tile([D, D], dtype=f32)
            nc.sync.dma_start(out=w2f[:], in_=w2_ext[:])
            w2 = cp.tile([D, D], dtype=bf16)
            nc.vector.tensor_copy(w2[:], w2f[:])
            b1 = cp.tile([1, D], dtype=f32)
            nc.sync.dma_start(out=b1[:], in_=b1_ext[:])
            b2 = cp.tile([1, D], dtype=f32)
            nc.sync.dma_start(out=b2[:], in_=b2_ext[:])
            ones = cp.tile([1, 512], dtype=f32)
            nc.sync.dma_start(out=ones[:], in_=ones_ext[:])
            invsA = cp.tile([P, NB], dtype=f32)
            nc.sync.dma_start(out=invsA[:], in_=invsA_ext[:])

            for ci, (b0, w) in enumerate(_chunks()):
                cw = w * P
                c0 = b0 * P
                xt = xp.tile([D, cw], dtype=f32)
                ldq = nc.sync if ci % 2 == 0 else nc.gpsimd
                ldq.dma_start(out=xt[:], in_=xT_ext[:, c0:c0 + cw])
                # y1 = x@W1 (+ b1), feature-major [D, cw]
                ps1 = pp1.tile([P, cw], dtype=f32, space="PSUM")
                nc.tensor.matmul(out=ps1[:], lhsT=w1[:], rhs=xt[:],
                                 start=True, stop=not bias_nz)
                if bias_nz:
                    nc.tensor.matmul(out=ps1[:], lhsT=b1[:],
                                     rhs=ones[:, :cw], start=False, stop=True)
                # lrelu: t01 = 0.01*y1 (Act), z1 = max(y1, t01) (DVE)
                t01 = sp.tile([P, cw], dtype=bf16)
                nc.scalar.mul(t01[:], ps1[:], 0.01)
                z1 = sp.tile([P, cw], dtype=bf16)
                nc.vector.tensor_tensor(out=z1[:], in0=ps1[:], in1=t01[:],
                                        op=mybir.AluOpType.max)
                # per 128-node block: y2 = z1_blk.T @ W2 (+ b2), node-major
                stg = gp.tile([P, cw], dtype=bf16)
                for j in range(w):
                    ps2 = pp2.tile([P, D], dtype=f32, space="PSUM")
                    nc.tensor.matmul(out=ps2[:], lhsT=z1[:, j * P:(j + 1) * P],
                                     rhs=w2[:], start=True, stop=not bias_nz)
                    if bias_nz:
                        nc.tensor.matmul(out=ps2[:], lhsT=b2[:],
                                         rhs=ones[:, :D], start=False, stop=True)
                    bcol = b0 + j
                    dst = stg[:, j * P:(j + 1) * P]
                    if j % 2 == 1:
                        nc.scalar.activation(
                            out=dst, in_=ps2[:],
                            func=mybir.ActivationFunctionType.Copy,
                            bias=0.0, scale=invsA[:, bcol:bcol + 1])
                    else:
                        sv = invsA[:, bcol:bcol + 1].to_broadcast([P, D])
                        nc.vector.tensor_tensor(out=dst, in0=ps2[:], in1=sv,
                                                op=mybir.AluOpType.mult)
                wrq = nc.gpsimd if ci % 2 == 0 else nc.sync
                wrq.dma_start(out=out_ext[:, c0:c0 + cw], in_=stg[:])
    nc.finalize()
    return nc


def _split_of(splits):
    which = np.empty(NB, np.int32)
    for s, (lo, hi) in enumerate(splits):
        which[lo:hi] = s
    return which


def _build_phase_b(KS, splits, bias_nz=True):
    from concourse import bacc, mybir, tile
    from concourse.masks import make_identity

    f32 = mybir.dt.float32
    bf16 = mybir.dt.bfloat16
    fp8 = mybir.dt.float8e4
    i16 = mybir.dt.int16
    nc = bacc.Bacc("TRN2", target_bir_lowering=False, debug=False)
    tab_exts = [nc.declare_dram_parameter(f"tab{s}", [TROWS, P], bf16,
                                          isOutput=False)
                for s in range(len(splits))]
    idx_ext = nc.declare_dram_parameter("idx", [P, NB * KS * 8], i16,
                                        isOutput=False)
    mask_ext = nc.declare_dram_parameter("mask", [P, NB * KS * P], fp8,
                                         isOutput=False)
    invr_ext = nc.declare_dram_parameter("invr", [P, NB], f32, isOutput=False)
    invs_ext = nc.declare_dram_parameter("invs", [P, NB], f32, isOutput=False)
    invris_ext = nc.declare_dram_parameter("invris", [P, NB], f32, isOutput=False)
    wd_ext = nc.declare_dram_parameter("Wd", [D, C], f32, isOutput=False)
    bd_ext = nc.declare_dram_parameter("bdrow", [1, C], f32, isOutput=False)
    ones_ext = nc.declare_dram_parameter("ones512", [1, 512], f32, isOutput=False)
    out_ext = nc.declare_dram_parameter("h2s", [P, NB * C], bf16, isOutput=True)

    which = _split_of(splits)
    WGRP = 16

    with tile.TileContext(nc) as tc:
        with (
            tc.tile_pool(name="const", bufs=1) as cp,
            tc.tile_pool(name="gat", bufs=6) as gp,
            tc.tile_pool(name="msk", bufs=4) as mp,
            tc.tile_pool(name="work", bufs=6) as sp,
            tc.tile_pool(name="stg", bufs=2) as op,
            tc.tile_pool(name="psA", bufs=4, space="PSUM") as ppA,
            tc.tile_pool(name="psT", bufs=2, space="PSUM") as ppT,
            tc.tile_pool(name="psO", bufs=2, space="PSUM") as ppO,
        ):
            idx = cp.tile([P, NB * KS * 8], dtype=i16)
            nc.sync.dma_start(out=idx[:], in_=idx_ext[:])
            invr = cp.tile([P, NB], dtype=f32)
            nc.sync.dma_start(out=invr[:], in_=invr_ext[:])
            invs = cp.tile([P, NB], dtype=f32)
            nc.sync.dma_start(out=invs[:], in_=invs_ext[:])
            invris = cp.tile([P, NB], dtype=f32)
            nc.sync.dma_start(out=invris[:], in_=invris_ext[:])
            wdf = cp.tile([D, C], dtype=f32)
            nc.sync.dma_start(out=wdf[:], in_=wd_ext[:])
            wd = cp.tile([D, C], dtype=bf16)
            nc.vector.tensor_copy(wd[:], wdf[:])
            bd = cp.tile([1, C], dtype=f32)
            nc.sync.dma_start(out=bd[:], in_=bd_ext[:])
            ones = cp.tile([1, 512], dtype=f32)
            nc.sync.dma_start(out=ones[:], in_=ones_ext[:])
            identb = cp.tile([P, P], dtype=bf16)
            make_identity(nc, identb[:])

            mk = None
            stg = None
            for b in range(NB):
                if b % 2 == 0:
                    nmk = min(2, NB - b)
                    mk = mp.tile([P, nmk * KS * P], dtype=fp8)
                    nc.sync.dma_start(
                        out=mk[:],
                        in_=mask_ext[:, b * KS * P:(b + nmk) * KS * P])
                if b % WGRP == 0:
                    nw = min(WGRP, NB - b)
                    stg = op.tile([P, nw * C], dtype=bf16)
                g = gp.tile([P, KS, P], dtype=bf16)
                nc.gpsimd.dma_gather(
                    out_ap=g[:].bitcast(f32), in_ap=tab_exts[which[b]][:].bitcast(f32),
                    idxs_ap=idx[:, b * KS * 8:(b + 1) * KS * 8],
                    num_idxs=KS * P, num_idxs_reg=KS * P, elem_size=P // 2)
                psA = ppA.tile([P, D], dtype=f32, space="PSUM")
                mo = (b % 2) * KS * P
                if not bias_nz:
                    # transposed scheme: aggT = sum_k g_k^T @ m_k  [feat, recv]
                    # lrelu commutes with the positive invr scale, which is
                    # merged with invs into the final per-receiver stage scale
                    for k in range(KS):
                        nc.tensor.matmul(
                            out=psA[:], lhsT=g[:, k, :],
                            rhs=mk[:, mo + k * P:mo + (k + 1) * P],
                            start=(k == 0), stop=(k == KS - 1))
                    za = sp.tile([P, D], dtype=bf16)
                    nc.scalar.copy(za[:], psA[:])
                    zb = sp.tile([P, D], dtype=bf16)
                    nc.vector.tensor_scalar_mul(zb[:], za[:], 0.01)
                    h = sp.tile([P, D], dtype=bf16)
                    nc.vector.tensor_tensor(out=h[:], in0=za[:], in1=zb[:],
                                            op=mybir.AluOpType.max)
                    psO = ppO.tile([P, C], dtype=f32, space="PSUM")
                    nc.tensor.matmul(out=psO[:], lhsT=h[:], rhs=wd[:],
                                     start=True, stop=True)
                    dst = stg[:, (b % WGRP) * C:(b % WGRP + 1) * C]
                    nc.vector.tensor_tensor(
                        out=dst, in0=psO[:],
                        in1=invris[:, b:b + 1].to_broadcast([P, C]),
                        op=mybir.AluOpType.mult)
                else:
                    for k in range(KS):
                        nc.tensor.matmul(
                            out=psA[:], lhsT=mk[:, mo + k * P:mo + (k + 1) * P],
                            rhs=g[:, k, :], start=(k == 0), stop=(k == KS - 1))
                    za = sp.tile([P, D], dtype=bf16)
                    nc.scalar.activation(out=za[:], in_=psA[:],
                                         func=mybir.ActivationFunctionType.Copy,
                                         bias=0.0, scale=invr[:, b:b + 1])
                    zb = sp.tile([P, D], dtype=bf16)
                    nc.vector.tensor_scalar_mul(zb[:], za[:], 0.01)
                    h = sp.tile([P, D], dtype=bf16)
                    nc.vector.tensor_tensor(out=h[:], in0=za[:], in1=zb[:],
                                            op=mybir.AluOpType.max)
                    psT = ppT.tile([P, D], dtype=bf16, space="PSUM")
                    nc.tensor.transpose(out=psT[:], in_=h[:], identity=identb[:])
                    hT = sp.tile([P, D], dtype=bf16)
                    if b % 2 == 0:
                        nc.scalar.copy(hT[:], psT[:])
                    else:
                        nc.vector.tensor_copy(hT[:], psT[:])
                    psO = ppO.tile([P, C], dtype=f32, space="PSUM")
                    nc.tensor.matmul(out=psO[:], lhsT=hT[:], rhs=wd[:],
                                     start=True, stop=False)
                    nc.tensor.matmul(out=psO[:], lhsT=ones[:, :D], rhs=bd[:],
                                     start=False, stop=True)
                    nc.vector.tensor_tensor(
                        out=stg[:, (b % WGRP) * C:(b % WGRP + 1) * C],
                        in0=psO[:], in1=invs[:, b:b + 1].to_broadcast([P, C]),
                        op=mybir.AluOpType.mult)
                if b % WGRP == WGRP - 1 or b == NB - 1:
                    w0 = (b // WGRP) * WGRP
                    nc.scalar.dma_start(
                        out=out_ext[:, w0 * C:(b + 1) * C],
                        in_=stg[:, :(b + 1 - w0) * C])
    nc.finalize()
    return nc


def _build_phase_c(KS, splits):
    from concourse import bacc, mybir, tile

    f32 = mybir.dt.float32
    bf16 = mybir.dt.bfloat16
    fp8 = mybir.dt.float8e4
    i16 = mybir.dt.int16
    nc = bacc.Bacc("TRN2", target_bir_lowering=False, debug=False)
    tab_exts = [nc.declare_dram_parameter(f"tab{s}", [TROWS, P], bf16,
                                          isOutput=False)
                for s in range(len(splits))]
    idx_ext = nc.declare_dram_parameter("idx", [P, NB * KS * 8], i16,
                                        isOutput=False)
    mask_ext = nc.declare_dram_parameter("mask", [P, NB * KS * P], fp8,
                                         isOutput=False)
    invr_ext = nc.declare_dram_parameter("invr", [P, NB], f32, isOutput=False)
    out_ext = nc.declare_dram_parameter("res", [P, NB * C], f32, isOutput=True)

    which = _split_of(splits)
    WGRP = 16

    with tile.TileContext(nc) as tc:
        with (
            tc.tile_pool(name="const", bufs=1) as cp,
            tc.tile_pool(name="gat", bufs=6) as gp,
            tc.tile_pool(name="msk", bufs=4) as mp,
            tc.tile_pool(name="work", bufs=6) as sp,
            tc.tile_pool(name="stg", bufs=2) as op,
            tc.tile_pool(name="psC", bufs=6, space="PSUM") as ppC,
        ):
            idx = cp.tile([P, NB * KS * 8], dtype=i16)
            nc.sync.dma_start(out=idx[:], in_=idx_ext[:])
            invr = cp.tile([P, NB], dtype=f32)
            nc.sync.dma_start(out=invr[:], in_=invr_ext[:])

            mk = None
            stg = None
            for b in range(NB):
                if b % 2 == 0:
                    nmk = min(2, NB - b)
                    mk = mp.tile([P, nmk * KS * P], dtype=fp8)
                    nc.sync.dma_start(
                        out=mk[:],
                        in_=mask_ext[:, b * KS * P:(b + nmk) * KS * P])
                if b % WGRP == 0:
                    nw = min(WGRP, NB - b)
                    stg = op.tile([P, nw * C], dtype=f32)
                g = gp.tile([P, KS, P], dtype=bf16)
                nc.gpsimd.dma_gather(
                    out_ap=g[:].bitcast(f32), in_ap=tab_exts[which[b]][:].bitcast(f32),
                    idxs_ap=idx[:, b * KS * 8:(b + 1) * KS * 8],
                    num_idxs=KS * P, num_idxs_reg=KS * P, elem_size=P // 2)
                psC = ppC.tile([P, C], dtype=f32, space="PSUM")
                mo = (b % 2) * KS * P
                for k in range(KS):
                    nc.tensor.matmul(
                        out=psC[:], lhsT=mk[:, mo + k * P:mo + (k + 1) * P],
                        rhs=g[:, k, 0:C], start=(k == 0), stop=(k == KS - 1))
                # softmax: ex = exp(agg*invr) (logits O(10), no max-subtract)
                ex = sp.tile([P, C], dtype=bf16)
                nc.scalar.activation(out=ex[:], in_=psC[:],
                                     func=mybir.ActivationFunctionType.Exp,
                                     scale=invr[:, b:b + 1])
                dn = sp.tile([P, 1], dtype=f32)
                nc.vector.reduce_sum(dn[:], ex[:], axis=mybir.AxisListType.X)
                rd = sp.tile([P, 1], dtype=f32)
                nc.vector.reciprocal(rd[:], dn[:])
                nc.vector.tensor_tensor(
                    out=stg[:, (b % WGRP) * C:(b % WGRP + 1) * C],
                    in0=ex[:], in1=rd[:].to_broadcast([P, C]),
                    op=mybir.AluOpType.mult)
                if b % WGRP == WGRP - 1 or b == NB - 1:
                    w0 = (b // WGRP) * WGRP
                    nc.scalar.dma_start(
                        out=out_ext[:, w0 * C:(b + 1) * C],
                        in_=stg[:, :(b + 1 - w0) * C])
    nc.finalize()
    return nc


# ------------------------------------------------------- host-side oracles

def _lrelu(v):
    return np.maximum(v, 0.01 * v)


def host_h1s_table(pre):
    """Expected full h1s table [NCORES*NPAD, D] in phase-A (p,b) row order."""
    xT = pre["xT"]
    out = np.zeros((NCORES * NPAD, D), np.float32)
    for c in range(NCORES):
        xc = xT[c].T
        h = _lrelu(xc @ pre["W1"] + pre["b1row"])
        h = h @ pre["W2"] + pre["b2row"]
        h = h * pre["invsA"][c].T.reshape(-1, 1)
        hpb = h.reshape(NB, P, D).transpose(1, 0, 2).reshape(NPAD, D)
        out[c * NPAD:(c + 1) * NPAD] = hpb
    return out


def host_agg(pre, tabs, dim):
    """Segment-sum using the halo tables (matches the device data path)."""
    KS = pre["KS"]
    which = _split_of(pre["splits"])
    mask = np.asarray(pre["mask"]).astype(np.float32).reshape(
        NCORES, P, NB, KS, P)
    out = np.zeros((NCORES, P, NB, dim), np.float32)
    for c in range(NCORES):
        for s, (lo, hi) in enumerate(pre["splits"]):
            uniq = pre["gids"][c][s]
            nodes = pre["sendnode"][c, lo:hi]
            local = np.zeros(nodes.shape, np.int64)
            valid = nodes >= 0
            local[valid] = np.searchsorted(uniq, nodes[valid])
            tabf = np.asarray(tabs[c][s]).astype(np.float32)
            g = tabf[local][:, :, :dim]                     # [nb, KSLOT, dim]
            g = g.reshape(hi - lo, KS, P, dim)              # [nb, k, lane, d]
            out[c, :, lo:hi] = np.einsum(
                "lbkq,bkld->qbd", mask[c, :, lo:hi], g)
    return out


def host_h2s_table(pre, tabsB):
    agg = host_agg(pre, tabsB, D)
    out = np.zeros((NCORES * NPAD, C), np.float32)
    for c in range(NCORES):
        if pre["bias_nz"]:
            v = agg[c] * pre["invr_blk"][c][:, :, None]
            h = _lrelu(v)
            y = h.reshape(-1, D) @ pre["Wd"] + pre["bdrow"]
            y = y.reshape(P, NB, C) * pre["invs_blk"][c][:, :, None]
        else:
            h = _lrelu(agg[c])
            y = (h.reshape(-1, D) @ pre["Wd"]).reshape(P, NB, C)
            y = y * (pre["invr_blk"][c] * pre["invs_blk"][c])[:, :, None]
        out[c * NPAD:(c + 1) * NPAD] = y.reshape(NPAD, C)
    return out


def host_final(pre, tabsC):
    agg = host_agg(pre, tabsC, C)
    res = np.zeros((NCORES, NPAD, C), np.float32)
    for c in range(NCORES):
        v = agg[c] * pre["invr_blk"][c][:, :, None]
        e = np.exp(v)
        res[c] = (e / e.sum(-1, keepdims=True)).reshape(NPAD, C)
    return res


def unshard(pre, res_list):
    out = np.zeros((N, C), np.float32)
    for c in range(NCORES):
        r = np.asarray(res_list[c], np.float32).reshape(P, NB, C)
        nid = pre["node_at"][c]                  # [NB, P]
        valid = nid >= 0
        out[nid[valid]] = r.transpose(1, 0, 2)[valid]
    return out


# ------------------------------------------------------------------ driver

_EXEC_TIMES = []


def _run(nc, in_maps):
    from concourse.bass_utils import run_bass_kernel_spmd
    res = run_bass_kernel_spmd(nc, in_maps, core_ids=list(range(NCORES)))
    if res.exec_time_ns is not None:
        _EXEC_TIMES.append(res.exec_time_ns)
    return res.results


def kernel(x, senders, receivers, W1, b1, W2, b2, Wd, bd):
    pre = preprocess(x, senders, receivers, W1, b1, W2, b2, Wd, bd)
    KS = pre["KS"]

    nc_a = _build_phase_a(pre["bias_nz"])
    res_a = _run(nc_a, maps_a(pre))
    h1s = np.concatenate(
        [np.asarray(r["h1s"]).reshape(NPAD, D) for r in res_a], axis=0)
    tabsB = build_halo_tabs(pre, h1s, pre["rowA"], D)

    nc_b = _build_phase_b(KS, pre["splits"], pre["bias_nz"])
    res_b = _run(nc_b, maps_b(pre, tabsB))
    h2s = np.concatenate(
        [np.asarray(r["h2s"]).reshape(NPAD, C) for r in res_b], axis=0)
    tabsC = build_halo_tabs(pre, h2s, pre["rowB"], C)

    nc_c = _build_phase_c(KS, pre["splits"])
    res_c = _run(nc_c, maps_c(pre, tabsC))
    return unshard(pre, [r["res"] for r in res_c])
